# revision 1
# baseline (speedup 1.0000x reference)
"""Multi-head causal self-attention (B=2, S=2048, D=1024, H=16) on 8 trn2 cores.

Sharding: 2-way data-parallel over batch x 4-way tensor-parallel over heads.
Core c handles batch b=c//4 and heads [4*(c%4), 4*(c%4)+4).

Per-core device program (fp32 data, fp32r matmuls -> ~2.9e-4 rel err):
  1. QKV projections from host-pre-transposed x^T and W^T shards.
     Q^T,K^T produced as [head-channel, token]; V as [token, channel] with a
     fused ones-column (softmax-denominator trick).
  2. Flash-style causal attention per (t-block 512, chunk of 2 heads):
     both heads' scores^T land in one [128, 2, 512] PSUM pair of banks, one
     batched exp (ScalarE, scale=1/8) serves both, diagonal tiles masked via
     a lower-triangle multiply, AV pairs trail two u-steps behind the scores
     so PE stays dense. AV accumulates into PSUM [65, 512] whose row 64 is
     the softmax denominator (ones column of V). Normalization happens after
     AV: denominator broadcast by a K=1 matmul, reciprocal_approx_fast, then
     one multiply into OT.
  3. o_proj partial product over this core's 256 v-dims, per t-block.
Host sums the 4 per-batch partials (the v-contraction all-reduce) and stacks.
"""

import numpy as np
from contextlib import ExitStack

import concourse.bass as bass
import concourse.bacc as bacc
import concourse.tile as tile
import concourse.mybir as mybir
from concourse.bass_utils import run_bass_kernel_spmd

F32 = mybir.dt.float32
F32R = mybir.dt.float32r
BF16 = mybir.dt.bfloat16
EXP = mybir.ActivationFunctionType.Exp

B, S, D = 2, 2048, 1024
H, HS = 16, 64
NCORES = 8
HPC = H // (NCORES // B)  # heads per core = 4
KD = HPC * HS             # per-core projected dims = 256
NKK = KD // 128           # head-dim partition chunks = 2
NDC = D // 128            # contraction chunks = 8
TB = 512                  # t-block width
NTB = S // TB             # 4
NUT = S // 128            # u-tiles = 16
SCALE = float(HS) ** -0.5


def build_program():
    rr = lambda ap: ap
    nc = bacc.Bacc("TRN2", target_bir_lowering=False, debug=False)
    xt = nc.dram_tensor("xt", [D, S], F32R, kind="ExternalInput").ap()
    wqt = nc.dram_tensor("wqt", [D, KD], F32R, kind="ExternalInput").ap()
    wkt = nc.dram_tensor("wkt", [D, KD], F32R, kind="ExternalInput").ap()
    wvt = nc.dram_tensor("wvt", [D, KD], F32R, kind="ExternalInput").ap()
    wot = nc.dram_tensor("wot", [KD, D], F32R, kind="ExternalInput").ap()
    maskd = nc.dram_tensor("mask", [128, 128], F32R, kind="ExternalInput").ap()
    seld = nc.dram_tensor("sel", [2, 128], F32R, kind="ExternalInput").ap()
    y = nc.dram_tensor("y", [S, D], F32, kind="ExternalOutput").ap()

    with tile.TileContext(nc) as tc, ExitStack() as ctx:
        wpool = ctx.enter_context(tc.tile_pool(name="w", bufs=1))
        big = ctx.enter_context(tc.tile_pool(name="big", bufs=1))
        xtg_pool = ctx.enter_context(tc.tile_pool(name="xtg", bufs=32))
        e_pool = ctx.enter_context(tc.tile_pool(name="expS", bufs=6))
        sm_pool = ctx.enter_context(tc.tile_pool(name="small", bufs=2))
        ypool = ctx.enter_context(tc.tile_pool(name="yout", bufs=3))

        # --- persistent SBUF tensors ---
        wq_sb = wpool.tile([128, NDC, KD], F32R)
        wk_sb = wpool.tile([128, NDC, KD], F32R)
        wv_sb = wpool.tile([128, NDC, KD], F32R)
        wo_sb = wpool.tile([128, NKK, D], F32R)
        mask_sb = wpool.tile([128, 128], F32R)
        selA_sb = wpool.tile([1, 128], F32R)
        selB_sb = wpool.tile([1, 128], F32R)
        QT = big.tile([128, NKK, S], F32R)   # [channel(2 heads), kk, token]
        KT = big.tile([128, NKK, S], F32R)
        VA = big.tile([128, HPC, NUT, HS + 1], F32R)  # [tok, head, utile, ch|1]
        OT = big.tile([128, NKK, S], F32R)   # normalized attention out^T

        nc.sync.dma_start(wq_sb[:], wqt.rearrange("(c p) k -> p c k", p=128))
        nc.sync.dma_start(wk_sb[:], wkt.rearrange("(c p) k -> p c k", p=128))
        nc.sync.dma_start(wv_sb[:], wvt.rearrange("(c p) k -> p c k", p=128))
        nc.sync.dma_start(wo_sb[:], wot.rearrange("(c p) d -> p c d", p=128))
        nc.sync.dma_start(mask_sb[:], maskd)
        nc.sync.dma_start(selA_sb[:], seld[0:1, :])
        nc.sync.dma_start(selB_sb[:], seld[1:2, :])
        # mask row 0 is all-ones (u=0 <= every t); col 127 likewise.
        nc.vector.tensor_copy(
            VA[:, :, :, HS], mask_sb[:, 127:128].to_broadcast([128, HPC, NUT])
        )

        # tg-major issue order: t-group 0's chunks arrive first so the first
        # Q/K accumulation chain isn't gated on the whole x^T transfer.
        xts = [[None] * NTB for _ in range(NDC)]
        for tg in range(NTB):
            for c in range(NDC):
                t = xtg_pool.tile([128, TB], F32R, tag="xtg", name=f"xt{c}_{tg}")
                nc.sync.dma_start(
                    t[:], xt[128 * c:128 * (c + 1), TB * tg:TB * (tg + 1)]
                )
                xts[c][tg] = t

        def qkv(tg, pool):
            """Projections for t-group tg: Q^T/K^T columns, V u-tiles."""
            for w_sb, dst in ((wq_sb, QT), (wk_sb, KT)):
                for kk in range(NKK):
                    ps = pool.tile([128, TB], F32, tag="m512", name="qk_ps")
                    for c in range(NDC):
                        nc.tensor.matmul(
                            ps[:],
                            rr(w_sb[:, c, 128 * kk:128 * (kk + 1)]),
                            rr(xts[c][tg][:]),
                            start=(c == 0), stop=(c == NDC - 1),
                        )
                    nc.scalar.copy(dst[:, kk, TB * tg:TB * (tg + 1)], ps[:])
            for tt in range(TB // 128):
                ps = pool.tile([128, KD], F32, tag="m512", name="v_ps")
                for c in range(NDC):
                    nc.tensor.matmul(
                        ps[:],
                        rr(xts[c][tg][:, 128 * tt:128 * (tt + 1)]),
                        rr(wv_sb[:, c, :]),
                        start=(c == 0), stop=(c == NDC - 1),
                    )
                ut = (TB // 128) * tg + tt
                for h in range(HPC):
                    nc.vector.tensor_copy(
                        VA[:, h, ut, 0:HS], ps[:, HS * h:HS * (h + 1)]
                    )

        def o_proj(tb):
            for i in range(4 * tb, 4 * tb + 4):
                for j in range(D // 512):
                    ps = psM.tile([128, 512], F32, tag="m512", name="yps")
                    for vc in range(NKK):
                        nc.tensor.matmul(
                            ps[:],
                            rr(OT[:, vc, 128 * i:128 * (i + 1)]),
                            rr(wo_sb[:, vc, 512 * j:512 * (j + 1)]),
                            start=(vc == 0), stop=(vc == NKK - 1),
                        )
                    yt = ypool.tile([128, 512], F32, tag="yt")
                    nc.vector.tensor_copy(yt[:], ps[:])
                    nc.sync.dma_start(
                        y[128 * i:128 * (i + 1), 512 * j:512 * (j + 1)], yt[:]
                    )

        def attention(tb, mid_hook=None):
            """Causal attention for t-block tb, heads paired per chunk, AV
            trailing one u-step behind scores. mid_hook emits deferred work
            (previous block's o_proj) after the hp=0 group so PE has scores
            in flight while the norm chain drains."""
            nut = 4 * tb + 4
            for hp in range(NKK):
                if hp == 1 and mid_hook is not None:
                    mid_hook()
                Os = [psO.tile([HS + 1, TB], F32, tag="av", name=f"O{g}")
                      for g in range(2)]

                def av_pair(pes, ptoff, pk, stop):
                    for g in range(2):
                        nc.tensor.matmul(
                            Os[g][:, ptoff:],
                            rr(VA[:, 2 * hp + g, pk, :]),
                            rr(pes[:, g, ptoff:]),
                            start=(pk == 0), stop=stop,
                        )

                pend = []
                for k in range(nut):
                    toff = max(0, 128 * (k - 4 * tb))
                    sp = psS.tile([128, 2, TB], F32, tag="sp")
                    for g in range(2):
                        nc.tensor.matmul(
                            sp[:, g, toff:],
                            rr(KT[64 * g:64 * g + 64, hp, 128 * k:128 * (k + 1)]),
                            rr(QT[64 * g:64 * g + 64, hp,
                                  TB * tb + toff:TB * (tb + 1)]),
                            start=True, stop=True,
                        )
                    es = e_pool.tile([128, 2, TB], F32R, tag="expS")
                    nc.scalar.activation(es[:, :, toff:], sp[:, :, toff:], EXP,
                                         scale=SCALE)
                    if k >= 4 * tb:  # diagonal: zero the u>t triangle
                        for g in range(2):
                            nc.vector.tensor_mul(
                                es[:, g, toff:toff + 128],
                                es[:, g, toff:toff + 128], mask_sb[:]
                            )
                    pend.append((es, toff, k))
                    if len(pend) > 2:
                        av_pair(*pend.pop(0), stop=False)
                for i, p in enumerate(pend):
                    av_pair(*p, stop=(i == len(pend) - 1))
                dens = [sm_pool.tile([1, TB], F32R, tag=f"den{g}",
                                     name=f"den{g}") for g in range(2)]
                for g in range(2):
                    nc.vector.tensor_copy(dens[g][:], Os[g][HS:HS + 1, :])
                bc_ps = psM.tile([128, TB], F32, tag="m512", name="bc_ps")
                nc.tensor.matmul(bc_ps[:], selA_sb[:], dens[0][:],
                                 start=True, stop=False)
                nc.tensor.matmul(bc_ps[:], selB_sb[:], dens[1][:],
                                 start=False, stop=True)
                bc = sm_pool.tile([128, TB], F32, tag="bc_sb")
                nc.vector.reciprocal_approx_fast(bc[:], bc_ps[:])
                for g in range(2):
                    ro = 64 * g
                    nc.vector.tensor_mul(
                        OT[ro:ro + 64, hp, TB * tb:TB * (tb + 1)],
                        Os[g][0:HS, :], bc[ro:ro + 64, :]
                    )

        with tc.tile_pool(name="ps1", bufs=6, space="PSUM") as ps1:
            for tg in range(NTB):
                qkv(tg, ps1)
        psS_ctx = tc.tile_pool(name="psS", bufs=2, space="PSUM")
        psS = psS_ctx.__enter__()
        psM_ctx = tc.tile_pool(name="psM", bufs=2, space="PSUM")
        psM = psM_ctx.__enter__()
        psO_ctx = tc.tile_pool(name="psO", bufs=2, space="PSUM")
        psO = psO_ctx.__enter__()
        for tb in range(NTB):
            attention(tb, mid_hook=(lambda p=tb - 1: o_proj(p)) if tb > 0
                      else None)
        o_proj(NTB - 1)
        psO_ctx.__exit__(None, None, None)
        psM_ctx.__exit__(None, None, None)
        psS_ctx.__exit__(None, None, None)

    nc.compile()
    return nc


def make_in_maps(x, q_w, k_w, v_w, o_w):
    x = np.asarray(x, dtype=np.float32)
    mask = np.triu(np.ones((128, 128), dtype=np.float32))  # keep where u <= t
    sel = np.zeros((2, 128), dtype=np.float32)
    sel[0, 0:64] = 1.0
    sel[1, 64:128] = 1.0
    xtb = [np.ascontiguousarray(x[b].T) for b in range(B)]
    in_maps = []
    for c in range(NCORES):
        b, hg = divmod(c, NCORES // B)
        sl = slice(hg * KD, (hg + 1) * KD)
        in_maps.append({
            "xt": xtb[b],
            "wqt": np.ascontiguousarray(np.asarray(q_w, np.float32)[sl, :].T),
            "wkt": np.ascontiguousarray(np.asarray(k_w, np.float32)[sl, :].T),
            "wvt": np.ascontiguousarray(np.asarray(v_w, np.float32)[sl, :].T),
            "wot": np.ascontiguousarray(np.asarray(o_w, np.float32)[:, sl].T),
            "mask": mask,
            "sel": sel,
        })
    return in_maps


def combine_outputs(results):
    """results: list of 8 dicts with per-core partial y [S, D]."""
    per_b = NCORES // B
    ys = [np.asarray(results[c]["y"], dtype=np.float32) for c in range(NCORES)]
    out = np.stack(
        [sum(ys[b * per_b + i] for i in range(per_b)) for b in range(B)]
    )
    return np.ascontiguousarray(out, dtype=np.float32)


_PROGRAM = None


def kernel(x, q_proj_weight, k_proj_weight, v_proj_weight, o_proj_weight,
           **extra):
    global _PROGRAM
    if _PROGRAM is None:
        _PROGRAM = build_program()
    in_maps = make_in_maps(x, q_proj_weight, k_proj_weight, v_proj_weight,
                           o_proj_weight)
    res = run_bass_kernel_spmd(_PROGRAM, in_maps, list(range(NCORES)))
    return combine_outputs(res.results)


if __name__ == "__main__":
    nc = build_program()
    n = len(nc.m.functions[0].blocks[0].instructions) if nc.m.functions else -1
    print("program built")



# revision 8
# speedup vs baseline: 1.1319x; 1.1319x over previous
"""Multi-head causal self-attention (B=2, S=2048, D=1024, H=16) on 8 trn2 cores.

Sharding: 2-way data-parallel over batch x 4-way tensor-parallel over heads.
Core c handles batch b=c//4 and heads [4*(c%4), 4*(c%4)+4).

v2 rewrite vs the 207us baseline:
  * bf16 data path everywhere (fp32 PSUM accumulation): halves DMA bytes,
    enables FWL weight loads, 2x DVE on elementwise.
  * Single software-pipelined phase: QKV projection chains and o_proj
    chains are emitted as "filler units" interleaved into the attention
    u-step loop, with deadlines derived from dataflow. PE stays dense (no
    HAM re-throttle), ScalarE exp (the 81us serial floor) overlaps PE work.
  * Denominator: reciprocal straight from PSUM, K=2 sel-matmul broadcast.
  * Staging copies split across Scalar (early, pre-exp) and Vector.

Per-core device program:
  1. QKV projections from host-pre-transposed x^T and W^T shards (bf16).
  2. Flash-style causal attention per (t-block 512, head-pair): both
     heads' scores^T in one [128,2,512] PSUM pair (row-group concurrent
     matmuls), one batched exp (ScalarE, scale=1/8) -> bf16, diagonal
     masked via lower-triangle multiply, AV trails two u-steps. AV
     accumulates [65,512] with row 64 = softmax denominator (ones column
     of V).
  3. o_proj partial product per t-tile; host sums 4 partials per batch.
"""

import numpy as np
from contextlib import ExitStack

import ml_dtypes

import concourse.bass as bass
import concourse.bacc as bacc
import concourse.tile as tile
import concourse.mybir as mybir
from concourse.bass_utils import run_bass_kernel_spmd

F32 = mybir.dt.float32
F32R = mybir.dt.float32r
BF16 = mybir.dt.bfloat16
EXP = mybir.ActivationFunctionType.Exp
BFNP = ml_dtypes.bfloat16

B, S, D = 2, 2048, 1024
H, HS = 16, 64
NCORES = 8
HPC = H // (NCORES // B)  # heads per core = 4
KD = HPC * HS             # per-core projected dims = 256
NKK = KD // 128           # head-pair chunks = 2
NDC = D // 128            # contraction chunks = 8
TB = 512                  # t-block width
NTB = S // TB             # 4
NUT = S // 128            # u-tiles = 16
NJ = D // TB              # o_proj column blocks = 2
SCALE = float(HS) ** -0.5


def build_program():
    nc = bacc.Bacc("TRN2", target_bir_lowering=False, debug=False)
    xt = nc.dram_tensor("xt", [NDC * NTB * 128, TB], BF16,
                        kind="ExternalInput").ap()
    wqt = nc.dram_tensor("wqt", [128, NDC * KD], BF16,
                         kind="ExternalInput").ap()
    wkt = nc.dram_tensor("wkt", [128, NDC * KD], BF16,
                         kind="ExternalInput").ap()
    wvt = nc.dram_tensor("wvt", [128, NDC * KD], BF16,
                         kind="ExternalInput").ap()
    wot = nc.dram_tensor("wot", [128, NKK * D], BF16,
                         kind="ExternalInput").ap()
    maskd = nc.dram_tensor("mask", [128, 128], BF16, kind="ExternalInput").ap()
    seld = nc.dram_tensor("sel", [2, 128], F32R, kind="ExternalInput").ap()
    y = nc.dram_tensor("y", [S, D], F32, kind="ExternalOutput").ap()

    with tile.TileContext(nc) as tc, ExitStack() as ctx:
        wpool = ctx.enter_context(tc.tile_pool(name="w", bufs=1))
        big = ctx.enter_context(tc.tile_pool(name="big", bufs=1))
        xtg_pool = ctx.enter_context(tc.tile_pool(name="xtg", bufs=32))
        e_pool = ctx.enter_context(tc.tile_pool(name="expS", bufs=6))
        sm_pool = ctx.enter_context(tc.tile_pool(name="small", bufs=2))
        ypool = ctx.enter_context(tc.tile_pool(name="yout", bufs=4))
        psS = ctx.enter_context(tc.tile_pool(name="psS", bufs=2, space="PSUM"))
        psM = ctx.enter_context(tc.tile_pool(name="psM", bufs=2, space="PSUM"))
        psO = ctx.enter_context(tc.tile_pool(name="psO", bufs=2, space="PSUM"))

        # --- persistent SBUF tensors ---
        wq_sb = wpool.tile([128, NDC, KD], BF16)
        wk_sb = wpool.tile([128, NDC, KD], BF16)
        wv_sb = wpool.tile([128, NDC, KD], BF16)
        wo_sb = wpool.tile([128, NKK, D], BF16)
        mask_sb = wpool.tile([128, 128], BF16)
        selA_sb = wpool.tile([1, 128], F32R)
        selB_sb = wpool.tile([1, 128], F32R)
        QT = big.tile([128, NKK, S], BF16)   # [channel(2 heads), kk, token]
        KT = big.tile([128, NKK, S], BF16)
        VA = big.tile([128, HPC, NUT, HS + 1], BF16)  # [tok, head, ut, ch|1]
        OT = big.tile([128, NKK, S], BF16)   # normalized attention out^T

        # DMA issue order is arrival priority: wq + x(tg0) gate the first
        # matmul; wk gates the first K chain; everything else trails.
        nc.sync.dma_start(wq_sb[:], wqt.rearrange("p (c k) -> p c k", c=NDC))
        xts = [[None] * NTB for _ in range(NDC)]
        for tg in range(NTB):
            for c in range(NDC):
                t = xtg_pool.tile([128, TB], BF16, tag="xtg",
                                  name=f"xt{c}_{tg}")
                r0 = (c * NTB + tg) * 128
                nc.sync.dma_start(t[:], xt[r0:r0 + 128, :])
                xts[c][tg] = t
            if tg == 0:
                nc.sync.dma_start(
                    wk_sb[:], wkt.rearrange("p (c k) -> p c k", c=NDC))
                nc.sync.dma_start(
                    wv_sb[:], wvt.rearrange("p (c k) -> p c k", c=NDC))
                nc.sync.dma_start(mask_sb[:], maskd)
                nc.sync.dma_start(selA_sb[:], seld[0:1, :])
                nc.sync.dma_start(selB_sb[:], seld[1:2, :])
        nc.sync.dma_start(wo_sb[:], wot.rearrange("p (v d) -> p v d", v=NKK))
        nc.vector.memset(VA[:, :, :, HS], 1.0)

        # --- filler units (each emits one PE chain + its staging op) ---
        def qk_unit(tg, kk, w_sb, dst, on_scalar):
            ps = psM.tile([128, TB], F32, tag="m512", name=f"qk{tg}_{kk}")
            for c in range(NDC):
                nc.tensor.matmul(
                    ps[:], w_sb[:, c, 128 * kk:128 * (kk + 1)], xts[c][tg][:],
                    start=(c == 0), stop=(c == NDC - 1),
                )
            sl = dst[:, kk, TB * tg:TB * (tg + 1)]
            if on_scalar:
                nc.scalar.copy(sl, ps[:])
            else:
                nc.vector.tensor_copy(sl, ps[:])

        def v_unit(tg, tt, on_scalar):
            ps = psM.tile([128, TB], F32, tag="m512", name=f"v{tg}_{tt}")
            for c in range(NDC):
                nc.tensor.matmul(
                    ps[:, 0:KD], xts[c][tg][:, 128 * tt:128 * (tt + 1)],
                    wv_sb[:, c, :], start=(c == 0), stop=(c == NDC - 1),
                )
            ut = (TB // 128) * tg + tt
            src = ps[:, 0:KD].rearrange("p (h c) -> p h c", h=HPC)
            if on_scalar:
                nc.scalar.copy(VA[:, :, ut, 0:HS], src)
            else:
                nc.vector.tensor_copy(VA[:, :, ut, 0:HS], src)

        def oproj_unit(tb, i, j):
            ps = psM.tile([128, TB], F32, tag="m512", name=f"y{i}_{j}")
            for vc in range(NKK):
                nc.tensor.matmul(
                    ps[:], OT[:, vc, 128 * i:128 * (i + 1)],
                    wo_sb[:, vc, TB * j:TB * (j + 1)],
                    start=(vc == 0), stop=(vc == NKK - 1),
                )
            yt = ypool.tile([128, TB], F32, tag="yt")
            nc.vector.tensor_copy(yt[:], ps[:])
            nc.sync.dma_start(
                y[128 * i:128 * (i + 1), TB * j:TB * (j + 1)], yt[:])

        # deadline table: (tb, hp, k) -> [unit closures]
        due = {}

        def add_due(tb, hp, k, fn):
            due.setdefault((tb, hp, k), []).append(fn)

        for tg in range(NTB):
            early = tg <= 1  # staging copies on ScalarE before exp saturates
            for kk in range(NKK):
                if (tg, kk) != (0, 0):
                    add_due(tg, kk, 0,
                            lambda tg=tg, kk=kk, e=early:
                            qk_unit(tg, kk, wq_sb, QT, e))
                    add_due(tg, kk, 4 * tg,
                            lambda tg=tg, kk=kk, e=early:
                            qk_unit(tg, kk, wk_sb, KT, e))
            for tt in range(TB // 128):
                add_due(tg, 0, 4 * tg + tt,
                        lambda tg=tg, tt=tt, e=early: v_unit(tg, tt, e))
        # o_proj(0) woven into (tb2, hp1); o_proj(1)/(2) into tb3
        for idx, (i, j) in enumerate((i, j) for i in range(0, 4)
                                     for j in range(NJ)):
            add_due(2, 1, (idx * 11) // 7, lambda i=i, j=j: oproj_unit(0, i, j))
        for idx, (i, j) in enumerate((i, j) for i in range(4, 8)
                                     for j in range(NJ)):
            add_due(3, 0, 2 * idx, lambda i=i, j=j: oproj_unit(1, i, j))
        for idx, (i, j) in enumerate((i, j) for i in range(8, 12)
                                     for j in range(NJ)):
            add_due(3, 1, 2 * idx, lambda i=i, j=j: oproj_unit(2, i, j))

        # --- prelude: first Q and K chains so scores(tb0) start ASAP ---
        qk_unit(0, 0, wq_sb, QT, True)
        qk_unit(0, 0, wk_sb, KT, True)

        # --- attention with interleaved filler ---
        for tb in range(NTB):
            nut = 4 * tb + 4
            for hp in range(NKK):
                Os = [psO.tile([HS + 1, TB], F32, tag="av", name=f"O{g}")
                      for g in range(2)]

                def av_pair(pes, ptoff, pk, stop):
                    for g in range(2):
                        nc.tensor.matmul(
                            Os[g][:, ptoff:],
                            VA[:, 2 * hp + g, pk, :],
                            pes[:, g, ptoff:],
                            start=(pk == 0), stop=stop,
                        )

                pend = []
                for k in range(nut):
                    for u in due.pop((tb, hp, k), []):
                        u()
                    if len(pend) > 2:
                        av_pair(*pend.pop(0), stop=False)
                    toff = max(0, 128 * (k - 4 * tb))
                    sp = psS.tile([128, 2, TB], F32, tag="sp")
                    for g in range(2):
                        nc.tensor.matmul(
                            sp[:, g, toff:],
                            KT[64 * g:64 * g + 64, hp, 128 * k:128 * (k + 1)],
                            QT[64 * g:64 * g + 64, hp,
                               TB * tb + toff:TB * (tb + 1)],
                            start=True, stop=True,
                        )
                    es = e_pool.tile([128, 2, TB], BF16, tag="expS")
                    nc.scalar.activation(es[:, :, toff:], sp[:, :, toff:], EXP,
                                         scale=SCALE)
                    if k >= 4 * tb:  # diagonal: zero the u>t triangle
                        for g in range(2):
                            nc.vector.tensor_mul(
                                es[:, g, toff:toff + 128],
                                es[:, g, toff:toff + 128], mask_sb[:]
                            )
                    pend.append((es, toff, k))
                for i, p in enumerate(pend):
                    av_pair(*p, stop=(i == len(pend) - 1))
                # normalization: recip of denominators (PSUM row 64),
                # K=2 matmul broadcast, then one multiply per head half.
                dens = [sm_pool.tile([1, TB], F32R, tag=f"den{g}",
                                     name=f"den{g}") for g in range(2)]
                for g in range(2):
                    nc.vector.tensor_copy(dens[g][:], Os[g][HS:HS + 1, :])
                bc_ps = psM.tile([128, TB], F32, tag="m512", name="bc_ps")
                nc.tensor.matmul(bc_ps[:], selA_sb[:], dens[0][:],
                                 start=True, stop=False)
                nc.tensor.matmul(bc_ps[:], selB_sb[:], dens[1][:],
                                 start=False, stop=True)
                bc = sm_pool.tile([128, TB], F32, tag="bc_sb")
                nc.vector.reciprocal_approx_fast(bc[:], bc_ps[:])
                for g in range(2):
                    ro = 64 * g
                    nc.vector.tensor_mul(
                        OT[ro:ro + 64, hp, TB * tb:TB * (tb + 1)],
                        Os[g][0:HS, :], bc[ro:ro + 64, :]
                    )
        # tail: last block's o_proj
        for i in range(12, 16):
            for j in range(NJ):
                oproj_unit(3, i, j)
        assert not due, f"unemitted filler units: {list(due)}"

    nc.compile()
    return nc


def make_in_maps(x, q_w, k_w, v_w, o_w):
    x = np.asarray(x, dtype=np.float32)
    mask = np.triu(np.ones((128, 128), dtype=np.float32))  # keep where u <= t
    sel = np.zeros((2, 128), dtype=np.float32)
    sel[0, 0:64] = 1.0
    sel[1, 64:128] = 1.0
    mask_bf = mask.astype(BFNP)

    def warr(w):  # [D_rows, cols] -> [128, nchunks*cols], chunked over rows
        d, cols = w.shape
        n = d // 128
        return np.ascontiguousarray(
            w.reshape(n, 128, cols).transpose(1, 0, 2).reshape(128, n * cols)
        ).astype(BFNP)

    xtb = []
    for b in range(B):
        xt = np.ascontiguousarray(x[b].T)  # [D, S]
        t = xt.reshape(NDC, 128, NTB, TB).transpose(0, 2, 1, 3)
        xtb.append(np.ascontiguousarray(
            t.reshape(NDC * NTB * 128, TB)).astype(BFNP))

    in_maps = []
    for c in range(NCORES):
        b, hg = divmod(c, NCORES // B)
        sl = slice(hg * KD, (hg + 1) * KD)
        in_maps.append({
            "xt": xtb[b],
            "wqt": warr(np.ascontiguousarray(np.asarray(q_w, np.float32)[sl, :].T)),
            "wkt": warr(np.ascontiguousarray(np.asarray(k_w, np.float32)[sl, :].T)),
            "wvt": warr(np.ascontiguousarray(np.asarray(v_w, np.float32)[sl, :].T)),
            "wot": warr(np.ascontiguousarray(np.asarray(o_w, np.float32)[:, sl].T)),
            "mask": mask_bf,
            "sel": sel,
        })
    return in_maps


def combine_outputs(results):
    """results: list of 8 dicts with per-core partial y [S, D]."""
    per_b = NCORES // B
    ys = [np.asarray(results[c]["y"], dtype=np.float32) for c in range(NCORES)]
    out = np.stack(
        [sum(ys[b * per_b + i] for i in range(per_b)) for b in range(B)]
    )
    return np.ascontiguousarray(out, dtype=np.float32)


_PROGRAM = None


def kernel(x, q_proj_weight, k_proj_weight, v_proj_weight, o_proj_weight,
           **extra):
    global _PROGRAM
    if _PROGRAM is None:
        _PROGRAM = build_program()
    in_maps = make_in_maps(x, q_proj_weight, k_proj_weight, v_proj_weight,
                           o_proj_weight)
    res = run_bass_kernel_spmd(_PROGRAM, in_maps, list(range(NCORES)))
    return combine_outputs(res.results)


if __name__ == "__main__":
    nc = build_program()
    print("program built")


# revision 13
# speedup vs baseline: 1.2939x; 1.1431x over previous
"""Multi-head causal self-attention (B=2, S=2048, D=1024, H=16) on 8 trn2 cores.

Sharding: 2-way data-parallel over batch x 4-way tensor-parallel over heads.
Core c handles batch b=c//4 and heads [4*(c%4), 4*(c%4)+4).

v2 rewrite vs the 207us baseline:
  * bf16 data path everywhere (fp32 PSUM accumulation): halves DMA bytes,
    enables FWL weight loads, 2x DVE on elementwise.
  * Single software-pipelined phase: QKV projection chains and o_proj
    chains are emitted as "filler units" interleaved into the attention
    u-step loop, with deadlines derived from dataflow. PE stays dense (no
    HAM re-throttle), ScalarE exp (the 81us serial floor) overlaps PE work.
  * Denominator: reciprocal straight from PSUM, K=2 sel-matmul broadcast.
  * Staging copies split across Scalar (early, pre-exp) and Vector.

Per-core device program:
  1. QKV projections from host-pre-transposed x^T and W^T shards (bf16).
  2. Flash-style causal attention per (t-block 512, head-pair): both
     heads' scores^T in one [128,2,512] PSUM pair (row-group concurrent
     matmuls), one batched exp (ScalarE, scale=1/8) -> bf16, diagonal
     masked via lower-triangle multiply, AV trails two u-steps. AV
     accumulates [65,512] with row 64 = softmax denominator (ones column
     of V).
  3. o_proj partial product per t-tile; host sums 4 partials per batch.
"""

import numpy as np
from contextlib import ExitStack

import ml_dtypes

import concourse.bass as bass
import concourse.bacc as bacc
import concourse.tile as tile
import concourse.mybir as mybir
from concourse.bass_utils import run_bass_kernel_spmd

F32 = mybir.dt.float32
F32R = mybir.dt.float32r
BF16 = mybir.dt.bfloat16
EXP = mybir.ActivationFunctionType.Exp
BFNP = ml_dtypes.bfloat16

B, S, D = 2, 2048, 1024
H, HS = 16, 64
NCORES = 8
HPC = H // (NCORES // B)  # heads per core = 4
KD = HPC * HS             # per-core projected dims = 256
NKK = KD // 128           # head-pair chunks = 2
NDC = D // 128            # contraction chunks = 8
TB = 512                  # t-block width
NTB = S // TB             # 4
NUT = S // 128            # u-tiles = 16
NJ = D // TB              # o_proj column blocks = 2
SCALE = float(HS) ** -0.5


def build_program():
    nc = bacc.Bacc("TRN2", target_bir_lowering=False, debug=False)
    xt = nc.dram_tensor("xt", [NDC * NTB * 128, TB], BF16,
                        kind="ExternalInput").ap()
    wqt = nc.dram_tensor("wqt", [128, NDC * KD], BF16,
                         kind="ExternalInput").ap()
    wkt = nc.dram_tensor("wkt", [128, NDC * KD], BF16,
                         kind="ExternalInput").ap()
    wvt = nc.dram_tensor("wvt", [128, NDC * KD], BF16,
                         kind="ExternalInput").ap()
    wot = nc.dram_tensor("wot", [128, NKK * D], BF16,
                         kind="ExternalInput").ap()
    maskd = nc.dram_tensor("mask", [128, 128], BF16, kind="ExternalInput").ap()
    seld = nc.dram_tensor("sel", [2, 128], F32R, kind="ExternalInput").ap()
    y = nc.dram_tensor("y", [S, D], BF16, kind="ExternalOutput").ap()

    with tile.TileContext(nc) as tc, ExitStack() as ctx:
        wpool = ctx.enter_context(tc.tile_pool(name="w", bufs=1))
        big = ctx.enter_context(tc.tile_pool(name="big", bufs=1))
        xtg_pool = ctx.enter_context(tc.tile_pool(name="xtg", bufs=32))
        e_pool = ctx.enter_context(tc.tile_pool(name="expS", bufs=6))
        sm_pool = ctx.enter_context(tc.tile_pool(name="small", bufs=2))
        ypool = ctx.enter_context(tc.tile_pool(name="yout", bufs=4))
        psS = ctx.enter_context(tc.tile_pool(name="psS", bufs=2, space="PSUM"))
        psM = ctx.enter_context(tc.tile_pool(name="psM", bufs=2, space="PSUM"))
        psO = ctx.enter_context(tc.tile_pool(name="psO", bufs=2, space="PSUM"))

        # --- persistent SBUF tensors ---
        wq_sb = wpool.tile([128, NDC, KD], BF16)
        wk_sb = wpool.tile([128, NDC, KD], BF16)
        wv_sb = wpool.tile([128, NDC, KD], BF16)
        wo_sb = wpool.tile([128, NKK, D], BF16)
        mask_sb = wpool.tile([128, 128], BF16)
        selA_sb = wpool.tile([1, 128], F32R)
        selB_sb = wpool.tile([1, 128], F32R)
        QT = big.tile([128, NKK, S], BF16)   # [channel(2 heads), kk, token]
        KT = big.tile([128, NKK, S], BF16)
        VA = big.tile([128, HPC, NUT, HS + 1], BF16)  # [tok, head, ut, ch|1]
        OT = big.tile([128, NKK, S], BF16)   # normalized attention out^T

        # DMA issue order is arrival priority: wq + x(tg0) gate the first
        # matmul; wk gates the first K chain; everything else trails.
        nc.sync.dma_start(wq_sb[:], wqt.rearrange("p (c k) -> p c k", c=NDC))
        xts = [[None] * NTB for _ in range(NDC)]
        for tg in range(NTB):
            for c in range(NDC):
                t = xtg_pool.tile([128, TB], BF16, tag="xtg",
                                  name=f"xt{c}_{tg}")
                r0 = (c * NTB + tg) * 128
                nc.sync.dma_start(t[:], xt[r0:r0 + 128, :])
                xts[c][tg] = t
            if tg == 0:
                nc.sync.dma_start(
                    wk_sb[:], wkt.rearrange("p (c k) -> p c k", c=NDC))
                nc.sync.dma_start(
                    wv_sb[:], wvt.rearrange("p (c k) -> p c k", c=NDC))
                nc.sync.dma_start(mask_sb[:], maskd)
                nc.sync.dma_start(selA_sb[:], seld[0:1, :])
                nc.sync.dma_start(selB_sb[:], seld[1:2, :])
        nc.sync.dma_start(wo_sb[:], wot.rearrange("p (v d) -> p v d", v=NKK))
        nc.vector.memset(VA[:, :, :, HS], 1.0)

        # --- filler units (each emits one PE chain + its staging op) ---
        def qk_unit(tg, kk, w_sb, dst, on_scalar):
            ps = psM.tile([128, TB], F32, tag="m512", name=f"qk{tg}_{kk}")
            for c in range(NDC):
                nc.tensor.matmul(
                    ps[:], w_sb[:, c, 128 * kk:128 * (kk + 1)], xts[c][tg][:],
                    start=(c == 0), stop=(c == NDC - 1),
                )
            sl = dst[:, kk, TB * tg:TB * (tg + 1)]
            if on_scalar:
                nc.scalar.copy(sl, ps[:])
            else:
                nc.vector.tensor_copy(sl, ps[:])

        def v_unit(tg, tt, on_scalar):
            ps = psM.tile([128, TB], F32, tag="m512", name=f"v{tg}_{tt}")
            for c in range(NDC):
                nc.tensor.matmul(
                    ps[:, 0:KD], xts[c][tg][:, 128 * tt:128 * (tt + 1)],
                    wv_sb[:, c, :], start=(c == 0), stop=(c == NDC - 1),
                )
            ut = (TB // 128) * tg + tt
            src = ps[:, 0:KD].rearrange("p (h c) -> p h c", h=HPC)
            if on_scalar:
                nc.scalar.copy(VA[:, :, ut, 0:HS], src)
            else:
                nc.vector.tensor_copy(VA[:, :, ut, 0:HS], src)

        def oproj_unit(tb, i, j):
            ps = psM.tile([128, TB], F32, tag="m512", name=f"y{i}_{j}")
            for vc in range(NKK):
                nc.tensor.matmul(
                    ps[:], OT[:, vc, 128 * i:128 * (i + 1)],
                    wo_sb[:, vc, TB * j:TB * (j + 1)],
                    start=(vc == 0), stop=(vc == NKK - 1),
                )
            yt = ypool.tile([128, TB], BF16, tag="yt")
            nc.vector.tensor_copy(yt[:], ps[:])
            nc.sync.dma_start(
                y[128 * i:128 * (i + 1), TB * j:TB * (j + 1)], yt[:])

        # deadline table: (tb, hp, k) -> [unit closures]
        due = {}

        def add_due(tb, hp, k, fn):
            due.setdefault((tb, hp, k), []).append(fn)

        for tg in range(NTB):
            early = tg <= 1  # staging copies on ScalarE before exp saturates
            for kk in range(NKK):
                if (tg, kk) != (0, 0):
                    # Q due 2 steps into the preceding phase so the chain
                    # never sits on the phase-boundary critical path.
                    q_tb, q_hp, q_k = (tg, 0, 2) if kk == 1 else (tg - 1, 1, 2)
                    add_due(q_tb, q_hp, q_k,
                            lambda tg=tg, kk=kk, e=early:
                            qk_unit(tg, kk, wq_sb, QT, e))
                    k_key = (0, 0, 3) if (tg, kk) == (0, 1) \
                        else (tg, kk, max(0, 4 * tg - 2))
                    add_due(*k_key,
                            lambda tg=tg, kk=kk, e=early:
                            qk_unit(tg, kk, wk_sb, KT, e))
            for tt in range(TB // 128):
                add_due(tg, 0, 4 * tg + tt,
                        lambda tg=tg, tt=tt, e=early: v_unit(tg, tt, e))
        # o_proj(0) woven into (tb2, hp1); o_proj(1)/(2) into tb3
        for idx, (i, j) in enumerate((i, j) for i in range(0, 4)
                                     for j in range(NJ)):
            add_due(2, 1, 2 + (idx * 9) // 7,
                    lambda i=i, j=j: oproj_unit(0, i, j))
        for idx, (i, j) in enumerate((i, j) for i in range(4, 8)
                                     for j in range(NJ)):
            key = (3, 0, 4 + 3 * idx) if idx < 4 else (3, 1, 2 + 3 * (idx - 4))
            add_due(*key, lambda i=i, j=j: oproj_unit(1, i, j))
        for idx, (i, j) in enumerate((i, j) for i in range(8, 12)
                                     for j in range(NJ)):
            add_due(3, 1, 4 + idx, lambda i=i, j=j: oproj_unit(2, i, j))

        # --- prelude: interleaved first Q/K chains so scores start ASAP ---
        psq = psM.tile([128, TB], F32, tag="m512", name="q00")
        psk = psM.tile([128, TB], F32, tag="m512", name="k00")
        for c in range(NDC):
            nc.tensor.matmul(psq[:], wq_sb[:, c, 0:128], xts[c][0][:],
                             start=(c == 0), stop=(c == NDC - 1))
            nc.tensor.matmul(psk[:], wk_sb[:, c, 0:128], xts[c][0][:],
                             start=(c == 0), stop=(c == NDC - 1))
        nc.scalar.copy(QT[:, 0, 0:TB], psq[:])
        nc.scalar.copy(KT[:, 0, 0:TB], psk[:])

        # --- attention: Scalar-paced u-steps; AV drain + normalization of
        # each phase deferred into the next phase (carry) so the boundary
        # never serializes PE->Scalar->DVE->PE. ---
        carry = [None]

        def close_phase(Os, pend, av_pair, tb, hp):
            def fin():
                for i, p in enumerate(pend):
                    av_pair(*p, stop=(i == len(pend) - 1))
                dens = [sm_pool.tile([1, TB], F32R, tag=f"den{g}",
                                     name=f"den{g}") for g in range(2)]
                for g in range(2):
                    nc.vector.tensor_copy(dens[g][:], Os[g][HS:HS + 1, :])
                bc_ps = psM.tile([128, TB], F32, tag="m512", name="bc_ps")
                nc.tensor.matmul(bc_ps[:], selA_sb[:], dens[0][:],
                                 start=True, stop=False)
                nc.tensor.matmul(bc_ps[:], selB_sb[:], dens[1][:],
                                 start=False, stop=True)
                bc = sm_pool.tile([128, TB], F32, tag="bc_sb")
                nc.vector.reciprocal_approx_fast(bc[:], bc_ps[:])
                for g in range(2):
                    ro = 64 * g
                    nc.vector.tensor_mul(
                        OT[ro:ro + 64, hp, TB * tb:TB * (tb + 1)],
                        Os[g][0:HS, :], bc[ro:ro + 64, :]
                    )
            return fin

        for tb in range(NTB):
            nut = 4 * tb + 4
            for hp in range(NKK):
                Os = [psO.tile([HS + 1, TB], F32, tag="av", name=f"O{g}")
                      for g in range(2)]

                def av_pair(pes, ptoff, pk, stop, Os=Os, hp=hp):
                    for g in range(2):
                        nc.tensor.matmul(
                            Os[g][:, ptoff:],
                            VA[:, 2 * hp + g, pk, :],
                            pes[:, g, ptoff:],
                            start=(pk == 0), stop=stop,
                        )

                pend = []
                for k in range(nut):
                    # filler units first: anything scores(k) might consume
                    # (KT/QT/VA producers) must precede it in the PE queue.
                    for u in due.pop((tb, hp, k), []):
                        u()
                    toff = max(0, 128 * (k - 4 * tb))
                    sp = psS.tile([128, 2, TB], F32, tag="sp")
                    for g in range(2):
                        nc.tensor.matmul(
                            sp[:, g, toff:],
                            KT[64 * g:64 * g + 64, hp, 128 * k:128 * (k + 1)],
                            QT[64 * g:64 * g + 64, hp,
                               TB * tb + toff:TB * (tb + 1)],
                            start=True, stop=True,
                        )
                    es = e_pool.tile([128, 2, TB], BF16, tag="expS")
                    nc.scalar.activation(es[:, :, toff:], sp[:, :, toff:], EXP,
                                         scale=SCALE)
                    if k >= 4 * tb:  # diagonal: zero the u>t triangle
                        for g in range(2):
                            nc.vector.tensor_mul(
                                es[:, g, toff:toff + 128],
                                es[:, g, toff:toff + 128], mask_sb[:]
                            )
                    if k == 0 and carry[0] is not None:
                        carry[0]()
                        carry[0] = None
                    if len(pend) > 2:
                        av_pair(*pend.pop(0), stop=False)
                    pend.append((es, toff, k))
                carry[0] = close_phase(Os, pend, av_pair, tb, hp)
        carry[0]()
        # tail: last block's o_proj
        for i in range(12, 16):
            for j in range(NJ):
                oproj_unit(3, i, j)
        assert not due, f"unemitted filler units: {list(due)}"

    nc.compile()
    return nc


def make_in_maps(x, q_w, k_w, v_w, o_w):
    x = np.asarray(x, dtype=np.float32)
    mask = np.triu(np.ones((128, 128), dtype=np.float32))  # keep where u <= t
    sel = np.zeros((2, 128), dtype=np.float32)
    sel[0, 0:64] = 1.0
    sel[1, 64:128] = 1.0
    mask_bf = mask.astype(BFNP)

    def warr(w):  # [D_rows, cols] -> [128, nchunks*cols], chunked over rows
        d, cols = w.shape
        n = d // 128
        return np.ascontiguousarray(
            w.reshape(n, 128, cols).transpose(1, 0, 2).reshape(128, n * cols)
        ).astype(BFNP)

    xtb = []
    for b in range(B):
        xt = np.ascontiguousarray(x[b].T)  # [D, S]
        t = xt.reshape(NDC, 128, NTB, TB).transpose(0, 2, 1, 3)
        xtb.append(np.ascontiguousarray(
            t.reshape(NDC * NTB * 128, TB)).astype(BFNP))

    in_maps = []
    for c in range(NCORES):
        b, hg = divmod(c, NCORES // B)
        sl = slice(hg * KD, (hg + 1) * KD)
        in_maps.append({
            "xt": xtb[b],
            "wqt": warr(np.ascontiguousarray(np.asarray(q_w, np.float32)[sl, :].T)),
            "wkt": warr(np.ascontiguousarray(np.asarray(k_w, np.float32)[sl, :].T)),
            "wvt": warr(np.ascontiguousarray(np.asarray(v_w, np.float32)[sl, :].T)),
            "wot": warr(np.ascontiguousarray(np.asarray(o_w, np.float32)[:, sl].T)),
            "mask": mask_bf,
            "sel": sel,
        })
    return in_maps


def combine_outputs(results):
    """results: list of 8 dicts with per-core partial y [S, D]."""
    per_b = NCORES // B
    ys = [np.asarray(results[c]["y"], dtype=np.float32) for c in range(NCORES)]
    out = np.stack(
        [sum(ys[b * per_b + i] for i in range(per_b)) for b in range(B)]
    )
    return np.ascontiguousarray(out, dtype=np.float32)


_PROGRAM = None


def kernel(x, q_proj_weight, k_proj_weight, v_proj_weight, o_proj_weight,
           **extra):
    global _PROGRAM
    if _PROGRAM is None:
        _PROGRAM = build_program()
    in_maps = make_in_maps(x, q_proj_weight, k_proj_weight, v_proj_weight,
                           o_proj_weight)
    res = run_bass_kernel_spmd(_PROGRAM, in_maps, list(range(NCORES)))
    return combine_outputs(res.results)


if __name__ == "__main__":
    nc = build_program()
    print("program built")


# revision 15
# speedup vs baseline: 1.3055x; 1.0089x over previous
"""Multi-head causal self-attention (B=2, S=2048, D=1024, H=16) on 8 trn2 cores.

Sharding: 2-way data-parallel over batch x 4-way tensor-parallel over heads.
Core c handles batch b=c//4 and heads [4*(c%4), 4*(c%4)+4).

v2 rewrite vs the 207us baseline:
  * bf16 data path everywhere (fp32 PSUM accumulation): halves DMA bytes,
    enables FWL weight loads, 2x DVE on elementwise.
  * Single software-pipelined phase: QKV projection chains and o_proj
    chains are emitted as "filler units" interleaved into the attention
    u-step loop, with deadlines derived from dataflow. PE stays dense (no
    HAM re-throttle), ScalarE exp (the 81us serial floor) overlaps PE work.
  * Denominator: reciprocal straight from PSUM, K=2 sel-matmul broadcast.
  * Staging copies split across Scalar (early, pre-exp) and Vector.

Per-core device program:
  1. QKV projections from host-pre-transposed x^T and W^T shards (bf16).
  2. Flash-style causal attention per (t-block 512, head-pair): both
     heads' scores^T in one [128,2,512] PSUM pair (row-group concurrent
     matmuls), one batched exp (ScalarE, scale=1/8) -> bf16, diagonal
     masked via lower-triangle multiply, AV trails two u-steps. AV
     accumulates [65,512] with row 64 = softmax denominator (ones column
     of V).
  3. o_proj partial product per t-tile; host sums 4 partials per batch.
"""

import numpy as np
from contextlib import ExitStack

import ml_dtypes

import concourse.bass as bass
import concourse.bacc as bacc
import concourse.tile as tile
import concourse.mybir as mybir
from concourse.bass_utils import run_bass_kernel_spmd

F32 = mybir.dt.float32
F32R = mybir.dt.float32r
BF16 = mybir.dt.bfloat16
EXP = mybir.ActivationFunctionType.Exp
BFNP = ml_dtypes.bfloat16

B, S, D = 2, 2048, 1024
H, HS = 16, 64
NCORES = 8
HPC = H // (NCORES // B)  # heads per core = 4
KD = HPC * HS             # per-core projected dims = 256
NKK = KD // 128           # head-pair chunks = 2
NDC = D // 128            # contraction chunks = 8
TB = 512                  # t-block width
NTB = S // TB             # 4
NUT = S // 128            # u-tiles = 16
NJ = D // TB              # o_proj column blocks = 2
SCALE = float(HS) ** -0.5


def build_program():
    nc = bacc.Bacc("TRN2", target_bir_lowering=False, debug=False)
    xt = nc.dram_tensor("xt", [NDC * NTB * 128, TB], BF16,
                        kind="ExternalInput").ap()
    wqt = nc.dram_tensor("wqt", [128, NDC * KD], BF16,
                         kind="ExternalInput").ap()
    wkt = nc.dram_tensor("wkt", [128, NDC * KD], BF16,
                         kind="ExternalInput").ap()
    wvt = nc.dram_tensor("wvt", [128, NDC * KD], BF16,
                         kind="ExternalInput").ap()
    wot = nc.dram_tensor("wot", [128, NKK * D], BF16,
                         kind="ExternalInput").ap()
    maskd = nc.dram_tensor("mask", [128, 128], BF16, kind="ExternalInput").ap()
    seld = nc.dram_tensor("sel", [2, 128], F32R, kind="ExternalInput").ap()
    y = nc.dram_tensor("y", [S, D], BF16, kind="ExternalOutput").ap()

    with tile.TileContext(nc) as tc, ExitStack() as ctx:
        wpool = ctx.enter_context(tc.tile_pool(name="w", bufs=1))
        big = ctx.enter_context(tc.tile_pool(name="big", bufs=1))
        xtg_pool = ctx.enter_context(tc.tile_pool(name="xtg", bufs=32))
        e_pool = ctx.enter_context(tc.tile_pool(name="expS", bufs=6))
        sm_pool = ctx.enter_context(tc.tile_pool(name="small", bufs=2))
        ypool = ctx.enter_context(tc.tile_pool(name="yout", bufs=4))
        psS = ctx.enter_context(tc.tile_pool(name="psS", bufs=2, space="PSUM"))
        psM = ctx.enter_context(tc.tile_pool(name="psM", bufs=2, space="PSUM"))
        psO = ctx.enter_context(tc.tile_pool(name="psO", bufs=2, space="PSUM"))

        # --- persistent SBUF tensors ---
        wq_sb = wpool.tile([128, NDC, KD], BF16)
        wk_sb = wpool.tile([128, NDC, KD], BF16)
        wv_sb = wpool.tile([128, NDC, KD], BF16)
        wo_sb = wpool.tile([128, NKK, D], BF16)
        mask_sb = wpool.tile([128, 128], BF16)
        selA_sb = wpool.tile([1, 128], F32R)
        selB_sb = wpool.tile([1, 128], F32R)
        QT = big.tile([128, NKK, S], BF16)   # [channel(2 heads), kk, token]
        KT = big.tile([128, NKK, S], BF16)
        VA = big.tile([128, HPC, NUT, HS + 1], BF16)  # [tok, head, ut, ch|1]
        OT = big.tile([128, NKK, S], BF16)   # normalized attention out^T

        # DMA issue order is arrival priority: wq + x(tg0) gate the first
        # matmul; wk gates the first K chain; everything else trails.
        nc.sync.dma_start(wq_sb[:], wqt.rearrange("p (c k) -> p c k", c=NDC))
        xts = [[None] * NTB for _ in range(NDC)]
        for tg in range(NTB):
            for c in range(NDC):
                t = xtg_pool.tile([128, TB], BF16, tag="xtg",
                                  name=f"xt{c}_{tg}")
                r0 = (c * NTB + tg) * 128
                nc.sync.dma_start(t[:], xt[r0:r0 + 128, :])
                xts[c][tg] = t
            if tg == 0:
                nc.sync.dma_start(
                    wk_sb[:], wkt.rearrange("p (c k) -> p c k", c=NDC))
                nc.sync.dma_start(
                    wv_sb[:], wvt.rearrange("p (c k) -> p c k", c=NDC))
                nc.sync.dma_start(mask_sb[:], maskd)
                nc.sync.dma_start(selA_sb[:], seld[0:1, :])
                nc.sync.dma_start(selB_sb[:], seld[1:2, :])
        nc.sync.dma_start(wo_sb[:], wot.rearrange("p (v d) -> p v d", v=NKK))
        nc.vector.memset(VA[:, :, :, HS], 1.0)

        # --- filler units, split into ~0.4-0.9us pieces so a single step's
        # injected PE work never exceeds the exp pacing budget. A unit's
        # PSUM tile is allocated by its first piece and released by the
        # staging copy in its last; pieces of one unit occupy consecutive
        # steps so at most two psM chains are ever open. ---
        def qk_pieces(tg, kk, w_sb, dst, on_scalar):
            st = {}

            def mk(half):
                def f():
                    if half == 0:
                        st['ps'] = psM.tile([128, TB], F32, tag="m512",
                                            name=f"qk{tg}_{kk}")
                    ps = st['ps']
                    for c in range(4 * half, 4 * half + 4):
                        nc.tensor.matmul(
                            ps[:], w_sb[:, c, 128 * kk:128 * (kk + 1)],
                            xts[c][tg][:],
                            start=(c == 0), stop=(c == NDC - 1),
                        )
                    if half == 1:
                        sl = dst[:, kk, TB * tg:TB * (tg + 1)]
                        if on_scalar:
                            nc.scalar.copy(sl, ps[:])
                        else:
                            nc.vector.tensor_copy(sl, ps[:])
                return f
            return [mk(0), mk(1)]

        def v_unit(tg, tt, on_scalar):
            ps = psM.tile([128, TB], F32, tag="m512", name=f"v{tg}_{tt}")
            for c in range(NDC):
                nc.tensor.matmul(
                    ps[:, 0:KD], xts[c][tg][:, 128 * tt:128 * (tt + 1)],
                    wv_sb[:, c, :], start=(c == 0), stop=(c == NDC - 1),
                )
            ut = (TB // 128) * tg + tt
            src = ps[:, 0:KD].rearrange("p (h c) -> p h c", h=HPC)
            if on_scalar:
                nc.scalar.copy(VA[:, :, ut, 0:HS], src)
            else:
                nc.vector.tensor_copy(VA[:, :, ut, 0:HS], src)

        def oproj_unit(tb, i, j, alt_pool=False):
            if alt_pool:
                spt = psS.tile([128, 2, TB], F32, tag="sp", name=f"y{i}_{j}")
                ps = spt[:, 0, :]
            else:
                ps = psM.tile([128, TB], F32, tag="m512", name=f"y{i}_{j}")[:]
            for vc in range(NKK):
                nc.tensor.matmul(
                    ps, OT[:, vc, 128 * i:128 * (i + 1)],
                    wo_sb[:, vc, TB * j:TB * (j + 1)],
                    start=(vc == 0), stop=(vc == NKK - 1),
                )
            yt = ypool.tile([128, TB], BF16, tag="yt")
            nc.vector.tensor_copy(yt[:], ps)
            nc.sync.dma_start(
                y[128 * i:128 * (i + 1), TB * j:TB * (j + 1)], yt[:])

        # deadline table: (tb, hp, k) -> [piece closures]
        due = {}

        def add_due(tb, hp, k, fn):
            due.setdefault((tb, hp, k), []).append(fn)

        def add_pieces(tb, hp, k_end, pieces):
            for i, p in enumerate(pieces):
                add_due(tb, hp, max(0, k_end - (len(pieces) - 1 - i)), p)

        for tg in range(NTB):
            early = tg <= 1  # staging copies on ScalarE before exp saturates
            for kk in range(NKK):
                if (tg, kk) != (0, 0):
                    q_key = (tg, 0, 4) if kk == 1 else (tg - 1, 1, 3)
                    if (tg, kk) == (0, 1):
                        q_key = (0, 0, 2)
                    add_pieces(*q_key, qk_pieces(tg, kk, wq_sb, QT, early))
                    k_key = (0, 0, 3) if (tg, kk) == (0, 1) \
                        else (tg, kk, 2)
                    add_pieces(*k_key, qk_pieces(tg, kk, wk_sb, KT, early))
            for tt in range(TB // 128):
                add_due(tg, 0, min(4 * tg + tt, 4 * tg + 3),
                        lambda tg=tg, tt=tt, e=early: v_unit(tg, tt, e))
        # o_proj(0) woven into (tb2, hp1); o_proj(1)/(2) into tb3
        for idx, (i, j) in enumerate((i, j) for i in range(0, 4)
                                     for j in range(NJ)):
            add_due(2, 1, 3 + idx, lambda i=i, j=j: oproj_unit(0, i, j))
        for idx, (i, j) in enumerate((i, j) for i in range(4, 8)
                                     for j in range(NJ)):
            key = (3, 0, 6 + 2 * idx) if idx < 4 else (3, 1, 3 + (idx - 4))
            add_due(*key, lambda i=i, j=j: oproj_unit(1, i, j))
        for idx, (i, j) in enumerate((i, j) for i in range(8, 12)
                                     for j in range(NJ)):
            add_due(3, 1, 7 + idx, lambda i=i, j=j: oproj_unit(2, i, j))

        # --- prelude: interleaved first Q/K chains so scores start ASAP ---
        psq = psM.tile([128, TB], F32, tag="m512", name="q00")
        psk = psM.tile([128, TB], F32, tag="m512", name="k00")
        for c in range(NDC):
            nc.tensor.matmul(psq[:], wq_sb[:, c, 0:128], xts[c][0][:],
                             start=(c == 0), stop=(c == NDC - 1))
            nc.tensor.matmul(psk[:], wk_sb[:, c, 0:128], xts[c][0][:],
                             start=(c == 0), stop=(c == NDC - 1))
        nc.scalar.copy(QT[:, 0, 0:TB], psq[:])
        nc.scalar.copy(KT[:, 0, 0:TB], psk[:])

        # --- attention: Scalar-paced u-steps; AV drain + normalization of
        # each phase deferred into the next phase (carry) so the boundary
        # never serializes PE->Scalar->DVE->PE. ---
        carry = [None]

        def close_phase(Os, pend, av_pair, tb, hp):
            def fin():
                for i, p in enumerate(pend):
                    av_pair(*p, stop=(i == len(pend) - 1))
                dens = [sm_pool.tile([1, TB], F32R, tag=f"den{g}",
                                     name=f"den{g}") for g in range(2)]
                for g in range(2):
                    nc.vector.tensor_copy(dens[g][:], Os[g][HS:HS + 1, :])
                bc_ps = psM.tile([128, TB], F32, tag="m512", name="bc_ps")
                nc.tensor.matmul(bc_ps[:], selA_sb[:], dens[0][:],
                                 start=True, stop=False)
                nc.tensor.matmul(bc_ps[:], selB_sb[:], dens[1][:],
                                 start=False, stop=True)
                bc = sm_pool.tile([128, TB], F32, tag="bc_sb")
                nc.vector.reciprocal_approx_fast(bc[:], bc_ps[:])
                for g in range(2):
                    ro = 64 * g
                    nc.vector.tensor_mul(
                        OT[ro:ro + 64, hp, TB * tb:TB * (tb + 1)],
                        Os[g][0:HS, :], bc[ro:ro + 64, :]
                    )
            return fin

        for tb in range(NTB):
            nut = 4 * tb + 4
            for hp in range(NKK):
                Os = [psO.tile([HS + 1, TB], F32, tag="av", name=f"O{g}")
                      for g in range(2)]

                def av_pair(pes, ptoff, pk, stop, Os=Os, hp=hp):
                    for g in range(2):
                        nc.tensor.matmul(
                            Os[g][:, ptoff:],
                            VA[:, 2 * hp + g, pk, :],
                            pes[:, g, ptoff:],
                            start=(pk == 0), stop=stop,
                        )

                pend = []
                for k in range(nut):
                    # filler units first: anything scores(k) might consume
                    # (KT/QT/VA producers) must precede it in the PE queue.
                    for u in due.pop((tb, hp, k), []):
                        u()
                    toff = max(0, 128 * (k - 4 * tb))
                    sp = psS.tile([128, 2, TB], F32, tag="sp")
                    for g in range(2):
                        nc.tensor.matmul(
                            sp[:, g, toff:],
                            KT[64 * g:64 * g + 64, hp, 128 * k:128 * (k + 1)],
                            QT[64 * g:64 * g + 64, hp,
                               TB * tb + toff:TB * (tb + 1)],
                            start=True, stop=True,
                        )
                    es = e_pool.tile([128, 2, TB], BF16, tag="expS")
                    nc.scalar.activation(es[:, :, toff:], sp[:, :, toff:], EXP,
                                         scale=SCALE)
                    if k >= 4 * tb:  # diagonal: zero the u>t triangle
                        for g in range(2):
                            nc.vector.tensor_mul(
                                es[:, g, toff:toff + 128],
                                es[:, g, toff:toff + 128], mask_sb[:]
                            )
                    if k == 0 and carry[0] is not None:
                        carry[0]()
                        carry[0] = None
                    if len(pend) > 2:
                        av_pair(*pend.pop(0), stop=False)
                    pend.append((es, toff, k))
                carry[0] = close_phase(Os, pend, av_pair, tb, hp)
        carry[0]()
        # tail: last block's o_proj, alternating PSUM pools so the
        # 2-deep psM rotation doesn't serialize the drain.
        for n, (i, j) in enumerate((i, j) for i in range(12, 16)
                                   for j in range(NJ)):
            oproj_unit(3, i, j, alt_pool=(n % 2 == 1))
        assert not due, f"unemitted filler units: {list(due)}"

    nc.compile()
    return nc


def make_in_maps(x, q_w, k_w, v_w, o_w):
    x = np.asarray(x, dtype=np.float32)
    mask = np.triu(np.ones((128, 128), dtype=np.float32))  # keep where u <= t
    sel = np.zeros((2, 128), dtype=np.float32)
    sel[0, 0:64] = 1.0
    sel[1, 64:128] = 1.0
    mask_bf = mask.astype(BFNP)

    def warr(w):  # [D_rows, cols] -> [128, nchunks*cols], chunked over rows
        d, cols = w.shape
        n = d // 128
        return np.ascontiguousarray(
            w.reshape(n, 128, cols).transpose(1, 0, 2).reshape(128, n * cols)
        ).astype(BFNP)

    xtb = []
    for b in range(B):
        xt = np.ascontiguousarray(x[b].T)  # [D, S]
        t = xt.reshape(NDC, 128, NTB, TB).transpose(0, 2, 1, 3)
        xtb.append(np.ascontiguousarray(
            t.reshape(NDC * NTB * 128, TB)).astype(BFNP))

    in_maps = []
    for c in range(NCORES):
        b, hg = divmod(c, NCORES // B)
        sl = slice(hg * KD, (hg + 1) * KD)
        in_maps.append({
            "xt": xtb[b],
            "wqt": warr(np.ascontiguousarray(np.asarray(q_w, np.float32)[sl, :].T)),
            "wkt": warr(np.ascontiguousarray(np.asarray(k_w, np.float32)[sl, :].T)),
            "wvt": warr(np.ascontiguousarray(np.asarray(v_w, np.float32)[sl, :].T)),
            "wot": warr(np.ascontiguousarray(np.asarray(o_w, np.float32)[:, sl].T)),
            "mask": mask_bf,
            "sel": sel,
        })
    return in_maps


def combine_outputs(results):
    """results: list of 8 dicts with per-core partial y [S, D]."""
    per_b = NCORES // B
    ys = [np.asarray(results[c]["y"], dtype=np.float32) for c in range(NCORES)]
    out = np.stack(
        [sum(ys[b * per_b + i] for i in range(per_b)) for b in range(B)]
    )
    return np.ascontiguousarray(out, dtype=np.float32)


_PROGRAM = None


def kernel(x, q_proj_weight, k_proj_weight, v_proj_weight, o_proj_weight,
           **extra):
    global _PROGRAM
    if _PROGRAM is None:
        _PROGRAM = build_program()
    in_maps = make_in_maps(x, q_proj_weight, k_proj_weight, v_proj_weight,
                           o_proj_weight)
    res = run_bass_kernel_spmd(_PROGRAM, in_maps, list(range(NCORES)))
    return combine_outputs(res.results)


if __name__ == "__main__":
    nc = build_program()
    print("program built")


# revision 17
# speedup vs baseline: 1.3207x; 1.0117x over previous
"""Multi-head causal self-attention (B=2, S=2048, D=1024, H=16) on 8 trn2 cores.

Sharding: 2-way data-parallel over batch x 4-way tensor-parallel over heads.
Core c handles batch b=c//4 and heads [4*(c%4), 4*(c%4)+4).

v2 rewrite vs the 207us baseline:
  * bf16 data path everywhere (fp32 PSUM accumulation): halves DMA bytes,
    enables FWL weight loads, 2x DVE on elementwise.
  * Single software-pipelined phase: QKV projection chains and o_proj
    chains are emitted as "filler units" interleaved into the attention
    u-step loop, with deadlines derived from dataflow. PE stays dense (no
    HAM re-throttle), ScalarE exp (the 81us serial floor) overlaps PE work.
  * Denominator: reciprocal straight from PSUM, K=2 sel-matmul broadcast.
  * Staging copies split across Scalar (early, pre-exp) and Vector.

Per-core device program:
  1. QKV projections from host-pre-transposed x^T and W^T shards (bf16).
  2. Flash-style causal attention per (t-block 512, head-pair): both
     heads' scores^T in one [128,2,512] PSUM pair (row-group concurrent
     matmuls), one batched exp (ScalarE, scale=1/8) -> bf16, diagonal
     masked via lower-triangle multiply, AV trails two u-steps. AV
     accumulates [65,512] with row 64 = softmax denominator (ones column
     of V).
  3. o_proj partial product per t-tile; host sums 4 partials per batch.
"""

import numpy as np
from contextlib import ExitStack

import ml_dtypes

import concourse.bass as bass
import concourse.bacc as bacc
import concourse.tile as tile
import concourse.mybir as mybir
from concourse.bass_utils import run_bass_kernel_spmd

F32 = mybir.dt.float32
F32R = mybir.dt.float32r
BF16 = mybir.dt.bfloat16
EXP = mybir.ActivationFunctionType.Exp
BFNP = ml_dtypes.bfloat16

B, S, D = 2, 2048, 1024
H, HS = 16, 64
NCORES = 8
HPC = H // (NCORES // B)  # heads per core = 4
KD = HPC * HS             # per-core projected dims = 256
NKK = KD // 128           # head-pair chunks = 2
NDC = D // 128            # contraction chunks = 8
TB = 512                  # t-block width
NTB = S // TB             # 4
NUT = S // 128            # u-tiles = 16
NJ = D // TB              # o_proj column blocks = 2
SCALE = float(HS) ** -0.5


def build_program():
    nc = bacc.Bacc("TRN2", target_bir_lowering=False, debug=False)
    xt = nc.dram_tensor("xt", [NDC * NTB * 128, TB], BF16,
                        kind="ExternalInput").ap()
    wqt = nc.dram_tensor("wqt", [128, NDC * KD], BF16,
                         kind="ExternalInput").ap()
    wkt = nc.dram_tensor("wkt", [128, NDC * KD], BF16,
                         kind="ExternalInput").ap()
    wvt = nc.dram_tensor("wvt", [128, NDC * KD], BF16,
                         kind="ExternalInput").ap()
    wot = nc.dram_tensor("wot", [128, NKK * D], BF16,
                         kind="ExternalInput").ap()
    maskd = nc.dram_tensor("mask", [128, 128], BF16, kind="ExternalInput").ap()
    seld = nc.dram_tensor("sel", [2, 128], F32R, kind="ExternalInput").ap()
    y = nc.dram_tensor("y", [S, D], BF16, kind="ExternalOutput").ap()

    with tile.TileContext(nc) as tc, ExitStack() as ctx:
        wpool = ctx.enter_context(tc.tile_pool(name="w", bufs=1))
        big = ctx.enter_context(tc.tile_pool(name="big", bufs=1))
        xtg_pool = ctx.enter_context(tc.tile_pool(name="xtg", bufs=32))
        e_pool = ctx.enter_context(tc.tile_pool(name="expS", bufs=6))
        sm_pool = ctx.enter_context(tc.tile_pool(name="small", bufs=2))
        ypool = ctx.enter_context(tc.tile_pool(name="yout", bufs=4))
        psS = ctx.enter_context(tc.tile_pool(name="psS", bufs=2, space="PSUM"))
        psM = ctx.enter_context(tc.tile_pool(name="psM", bufs=2, space="PSUM"))
        psO = ctx.enter_context(tc.tile_pool(name="psO", bufs=2, space="PSUM"))

        # --- persistent SBUF tensors ---
        wq_sb = wpool.tile([128, NDC, KD], BF16)
        wk_sb = wpool.tile([128, NDC, KD], BF16)
        wv_sb = wpool.tile([128, NDC, KD], BF16)
        wo_sb = wpool.tile([128, NKK, D], BF16)
        mask_sb = wpool.tile([128, 128], BF16)
        selA_sb = wpool.tile([1, 128], F32R)
        selB_sb = wpool.tile([1, 128], F32R)
        QT = big.tile([128, NKK, S], BF16)   # [channel(2 heads), kk, token]
        KT = big.tile([128, NKK, S], BF16)
        VA = big.tile([128, HPC, NUT, HS + 1], BF16)  # [tok, head, ut, ch|1]
        OT = big.tile([128, NKK, S], BF16)   # normalized attention out^T

        # DMA issue order is arrival priority: wq + x(tg0) gate the first
        # matmul; wk gates the first K chain; everything else trails.
        nc.sync.dma_start(wq_sb[:], wqt.rearrange("p (c k) -> p c k", c=NDC))
        xts = [[None] * NTB for _ in range(NDC)]
        for tg in range(NTB):
            for c in range(NDC):
                t = xtg_pool.tile([128, TB], BF16, tag="xtg",
                                  name=f"xt{c}_{tg}")
                r0 = (c * NTB + tg) * 128
                nc.sync.dma_start(t[:], xt[r0:r0 + 128, :])
                xts[c][tg] = t
            if tg == 0:
                nc.sync.dma_start(
                    wk_sb[:], wkt.rearrange("p (c k) -> p c k", c=NDC))
                nc.sync.dma_start(
                    wv_sb[:], wvt.rearrange("p (c k) -> p c k", c=NDC))
                nc.sync.dma_start(mask_sb[:], maskd)
                nc.sync.dma_start(selA_sb[:], seld[0:1, :])
                nc.sync.dma_start(selB_sb[:], seld[1:2, :])
        nc.sync.dma_start(wo_sb[:], wot.rearrange("p (v d) -> p v d", v=NKK))
        nc.vector.memset(VA[:, :, :, HS], 1.0)

        # --- filler units, split into ~0.4-0.9us pieces so a single step's
        # injected PE work never exceeds the exp pacing budget. A unit's
        # PSUM tile is allocated by its first piece and released by the
        # staging copy in its last; pieces of one unit occupy consecutive
        # steps so at most two psM chains are ever open. ---
        def qk_pieces(tg, kk, w_sb, dst, on_scalar):
            st = {}

            def mk(half):
                def f():
                    if half == 0:
                        st['ps'] = psM.tile([128, TB], F32, tag="m512",
                                            name=f"qk{tg}_{kk}")
                    ps = st['ps']
                    for c in range(4 * half, 4 * half + 4):
                        nc.tensor.matmul(
                            ps[:], w_sb[:, c, 128 * kk:128 * (kk + 1)],
                            xts[c][tg][:],
                            start=(c == 0), stop=(c == NDC - 1),
                        )
                    if half == 1:
                        sl = dst[:, kk, TB * tg:TB * (tg + 1)]
                        if on_scalar:
                            nc.scalar.copy(sl, ps[:])
                        else:
                            nc.vector.tensor_copy(sl, ps[:])
                return f
            return [mk(0), mk(1)]

        def v_unit(tg, tt, on_scalar):
            ps = psM.tile([128, TB], F32, tag="m512", name=f"v{tg}_{tt}")
            for c in range(NDC):
                nc.tensor.matmul(
                    ps[:, 0:KD], xts[c][tg][:, 128 * tt:128 * (tt + 1)],
                    wv_sb[:, c, :], start=(c == 0), stop=(c == NDC - 1),
                )
            ut = (TB // 128) * tg + tt
            src = ps[:, 0:KD].rearrange("p (h c) -> p h c", h=HPC)
            if on_scalar:
                nc.scalar.copy(VA[:, :, ut, 0:HS], src)
            else:
                nc.vector.tensor_copy(VA[:, :, ut, 0:HS], src)

        def oproj_unit(tb, i, j, alt_pool=False):
            if alt_pool:
                spt = psS.tile([128, 2, TB], F32, tag="sp", name=f"y{i}_{j}")
                ps = spt[:, 0, :]
            else:
                ps = psM.tile([128, TB], F32, tag="m512", name=f"y{i}_{j}")[:]
            for vc in range(NKK):
                nc.tensor.matmul(
                    ps, OT[:, vc, 128 * i:128 * (i + 1)],
                    wo_sb[:, vc, TB * j:TB * (j + 1)],
                    start=(vc == 0), stop=(vc == NKK - 1),
                )
            yt = ypool.tile([128, TB], BF16, tag="yt")
            nc.vector.tensor_copy(yt[:], ps)
            nc.sync.dma_start(
                y[128 * i:128 * (i + 1), TB * j:TB * (j + 1)], yt[:])

        # deadline table: (tb, hp, k) -> [piece closures]
        due = {}

        def add_due(tb, hp, k, fn):
            due.setdefault((tb, hp, k), []).append(fn)

        def add_pieces(tb, hp, k_end, pieces):
            for i, p in enumerate(pieces):
                add_due(tb, hp, max(0, k_end - (len(pieces) - 1 - i)), p)

        for tg in range(NTB):
            early = tg <= 1  # staging copies on ScalarE before exp saturates
            nut_tg = 4 * tg + 4
            for kk in range(NKK):
                if (tg, kk) != (0, 0):
                    q_key = (tg, 0, 4) if kk == 1 else \
                        (tg - 1, 1, min(4 * (tg - 1) + 3, 5))
                    if (tg, kk) == (0, 1):
                        q_key = (0, 0, 2)
                    add_pieces(*q_key, qk_pieces(tg, kk, wq_sb, QT, early))
                    k_key = (0, 0, 3) if (tg, kk) == (0, 1) \
                        else (tg, kk, max(2, 4 * tg - 2))
                    add_pieces(*k_key, qk_pieces(tg, kk, wk_sb, KT, early))
            # V(tg,0) is consumed in-phase; V(tg,1..3) only by the carried
            # AV drain, which runs at (tg, 1, k=1) — spread them there.
            vdl = [(tg, 0, min(4 * tg, nut_tg - 2)), (tg, 0, nut_tg - 1),
                   (tg, 1, 0), (tg, 1, 1)]
            for tt in range(TB // 128):
                add_due(*vdl[tt],
                        lambda tg=tg, tt=tt, e=early: v_unit(tg, tt, e))
        # o_proj(0) woven into (tb2, hp1); o_proj(1)/(2) into tb3
        for idx, (i, j) in enumerate((i, j) for i in range(0, 4)
                                     for j in range(NJ)):
            add_due(2, 1, 3 + idx, lambda i=i, j=j: oproj_unit(0, i, j))
        for idx, (i, j) in enumerate((i, j) for i in range(4, 8)
                                     for j in range(NJ)):
            key = (3, 0, 6 + 2 * idx) if idx < 4 else (3, 1, 3 + (idx - 4))
            add_due(*key, lambda i=i, j=j: oproj_unit(1, i, j))
        for idx, (i, j) in enumerate((i, j) for i in range(8, 12)
                                     for j in range(NJ)):
            add_due(3, 1, 7 + idx, lambda i=i, j=j: oproj_unit(2, i, j))

        # --- prelude: interleaved first Q/K chains so scores start ASAP ---
        psq = psM.tile([128, TB], F32, tag="m512", name="q00")
        psk = psM.tile([128, TB], F32, tag="m512", name="k00")
        for c in range(NDC):
            nc.tensor.matmul(psq[:], wq_sb[:, c, 0:128], xts[c][0][:],
                             start=(c == 0), stop=(c == NDC - 1))
            nc.tensor.matmul(psk[:], wk_sb[:, c, 0:128], xts[c][0][:],
                             start=(c == 0), stop=(c == NDC - 1))
        nc.scalar.copy(QT[:, 0, 0:TB], psq[:])
        nc.scalar.copy(KT[:, 0, 0:TB], psk[:])

        # --- attention: Scalar-paced u-steps; AV drain + normalization of
        # each phase deferred into the next phase (carry) so the boundary
        # never serializes PE->Scalar->DVE->PE. ---
        carry = [None]

        def close_phase(Os, pend, av_pair, tb, hp):
            def fin():
                for i, p in enumerate(pend):
                    av_pair(*p, stop=(i == len(pend) - 1))
                dens = [sm_pool.tile([1, TB], F32R, tag=f"den{g}",
                                     name=f"den{g}") for g in range(2)]
                for g in range(2):
                    nc.vector.tensor_copy(dens[g][:], Os[g][HS:HS + 1, :])
                bc_ps = psM.tile([128, TB], F32, tag="m512", name="bc_ps")
                nc.tensor.matmul(bc_ps[:], selA_sb[:], dens[0][:],
                                 start=True, stop=False)
                nc.tensor.matmul(bc_ps[:], selB_sb[:], dens[1][:],
                                 start=False, stop=True)
                bc = sm_pool.tile([128, TB], F32, tag="bc_sb")
                nc.vector.reciprocal_approx_fast(bc[:], bc_ps[:])
                for g in range(2):
                    ro = 64 * g
                    nc.vector.tensor_mul(
                        OT[ro:ro + 64, hp, TB * tb:TB * (tb + 1)],
                        Os[g][0:HS, :], bc[ro:ro + 64, :]
                    )
            return fin

        for tb in range(NTB):
            nut = 4 * tb + 4
            for hp in range(NKK):
                Os = [psO.tile([HS + 1, TB], F32, tag="av", name=f"O{g}")
                      for g in range(2)]

                def av_pair(pes, ptoff, pk, stop, Os=Os, hp=hp):
                    for g in range(2):
                        nc.tensor.matmul(
                            Os[g][:, ptoff:],
                            VA[:, 2 * hp + g, pk, :],
                            pes[:, g, ptoff:],
                            start=(pk == 0), stop=stop,
                        )

                pend = []
                for k in range(nut):
                    # filler units first: anything scores(k) might consume
                    # (KT/QT/VA producers) must precede it in the PE queue.
                    for u in due.pop((tb, hp, k), []):
                        u()
                    toff = max(0, 128 * (k - 4 * tb))
                    sp = psS.tile([128, 2, TB], F32, tag="sp")
                    for g in range(2):
                        nc.tensor.matmul(
                            sp[:, g, toff:],
                            KT[64 * g:64 * g + 64, hp, 128 * k:128 * (k + 1)],
                            QT[64 * g:64 * g + 64, hp,
                               TB * tb + toff:TB * (tb + 1)],
                            start=True, stop=True,
                        )
                    es = e_pool.tile([128, 2, TB], BF16, tag="expS")
                    nc.scalar.activation(es[:, :, toff:], sp[:, :, toff:], EXP,
                                         scale=SCALE)
                    if k >= 4 * tb:  # diagonal: zero the u>t triangle
                        for g in range(2):
                            nc.vector.tensor_mul(
                                es[:, g, toff:toff + 128],
                                es[:, g, toff:toff + 128], mask_sb[:]
                            )
                    if k == 1 and carry[0] is not None:
                        carry[0]()
                        carry[0] = None
                    if len(pend) > 2:
                        av_pair(*pend.pop(0), stop=False)
                    pend.append((es, toff, k))
                carry[0] = close_phase(Os, pend, av_pair, tb, hp)
        carry[0]()
        # tail: last block's o_proj, alternating PSUM pools so the
        # 2-deep psM rotation doesn't serialize the drain.
        for n, (i, j) in enumerate((i, j) for i in range(12, 16)
                                   for j in range(NJ)):
            oproj_unit(3, i, j, alt_pool=(n % 2 == 1))
        assert not due, f"unemitted filler units: {list(due)}"

    nc.compile()
    return nc


def make_in_maps(x, q_w, k_w, v_w, o_w):
    x = np.asarray(x, dtype=np.float32)
    mask = np.triu(np.ones((128, 128), dtype=np.float32))  # keep where u <= t
    sel = np.zeros((2, 128), dtype=np.float32)
    sel[0, 0:64] = 1.0
    sel[1, 64:128] = 1.0
    mask_bf = mask.astype(BFNP)

    def warr(w):  # [D_rows, cols] -> [128, nchunks*cols], chunked over rows
        d, cols = w.shape
        n = d // 128
        return np.ascontiguousarray(
            w.reshape(n, 128, cols).transpose(1, 0, 2).reshape(128, n * cols)
        ).astype(BFNP)

    xtb = []
    for b in range(B):
        xt = np.ascontiguousarray(x[b].T)  # [D, S]
        t = xt.reshape(NDC, 128, NTB, TB).transpose(0, 2, 1, 3)
        xtb.append(np.ascontiguousarray(
            t.reshape(NDC * NTB * 128, TB)).astype(BFNP))

    in_maps = []
    for c in range(NCORES):
        b, hg = divmod(c, NCORES // B)
        sl = slice(hg * KD, (hg + 1) * KD)
        in_maps.append({
            "xt": xtb[b],
            "wqt": warr(np.ascontiguousarray(np.asarray(q_w, np.float32)[sl, :].T)),
            "wkt": warr(np.ascontiguousarray(np.asarray(k_w, np.float32)[sl, :].T)),
            "wvt": warr(np.ascontiguousarray(np.asarray(v_w, np.float32)[sl, :].T)),
            "wot": warr(np.ascontiguousarray(np.asarray(o_w, np.float32)[:, sl].T)),
            "mask": mask_bf,
            "sel": sel,
        })
    return in_maps


def combine_outputs(results):
    """results: list of 8 dicts with per-core partial y [S, D]."""
    per_b = NCORES // B
    ys = [np.asarray(results[c]["y"], dtype=np.float32) for c in range(NCORES)]
    out = np.stack(
        [sum(ys[b * per_b + i] for i in range(per_b)) for b in range(B)]
    )
    return np.ascontiguousarray(out, dtype=np.float32)


_PROGRAM = None


def kernel(x, q_proj_weight, k_proj_weight, v_proj_weight, o_proj_weight,
           **extra):
    global _PROGRAM
    if _PROGRAM is None:
        _PROGRAM = build_program()
    in_maps = make_in_maps(x, q_proj_weight, k_proj_weight, v_proj_weight,
                           o_proj_weight)
    res = run_bass_kernel_spmd(_PROGRAM, in_maps, list(range(NCORES)))
    return combine_outputs(res.results)


if __name__ == "__main__":
    nc = build_program()
    print("program built")


# revision 18
# speedup vs baseline: 1.3362x; 1.0117x over previous
"""Multi-head causal self-attention (B=2, S=2048, D=1024, H=16) on 8 trn2 cores.

Sharding: 2-way data-parallel over batch x 4-way tensor-parallel over heads.
Core c handles batch b=c//4 and heads [4*(c%4), 4*(c%4)+4).

v2 rewrite vs the 207us baseline:
  * bf16 data path everywhere (fp32 PSUM accumulation): halves DMA bytes,
    enables FWL weight loads, 2x DVE on elementwise.
  * Single software-pipelined phase: QKV projection chains and o_proj
    chains are emitted as "filler units" interleaved into the attention
    u-step loop, with deadlines derived from dataflow. PE stays dense (no
    HAM re-throttle), ScalarE exp (the 81us serial floor) overlaps PE work.
  * Denominator: reciprocal straight from PSUM, K=2 sel-matmul broadcast.
  * Staging copies split across Scalar (early, pre-exp) and Vector.

Per-core device program:
  1. QKV projections from host-pre-transposed x^T and W^T shards (bf16).
  2. Flash-style causal attention per (t-block 512, head-pair): both
     heads' scores^T in one [128,2,512] PSUM pair (row-group concurrent
     matmuls), one batched exp (ScalarE, scale=1/8) -> bf16, diagonal
     masked via lower-triangle multiply, AV trails two u-steps. AV
     accumulates [65,512] with row 64 = softmax denominator (ones column
     of V).
  3. o_proj partial product per t-tile; host sums 4 partials per batch.
"""

import numpy as np
from contextlib import ExitStack

import ml_dtypes

import concourse.bass as bass
import concourse.bacc as bacc
import concourse.tile as tile
import concourse.mybir as mybir
from concourse.bass_utils import run_bass_kernel_spmd

F32 = mybir.dt.float32
F32R = mybir.dt.float32r
BF16 = mybir.dt.bfloat16
EXP = mybir.ActivationFunctionType.Exp
BFNP = ml_dtypes.bfloat16

B, S, D = 2, 2048, 1024
H, HS = 16, 64
NCORES = 8
HPC = H // (NCORES // B)  # heads per core = 4
KD = HPC * HS             # per-core projected dims = 256
NKK = KD // 128           # head-pair chunks = 2
NDC = D // 128            # contraction chunks = 8
TB = 512                  # t-block width
NTB = S // TB             # 4
NUT = S // 128            # u-tiles = 16
NJ = D // TB              # o_proj column blocks = 2
SCALE = float(HS) ** -0.5


def build_program():
    nc = bacc.Bacc("TRN2", target_bir_lowering=False, debug=False)
    xt = nc.dram_tensor("xt", [NDC * NTB * 128, TB], BF16,
                        kind="ExternalInput").ap()
    wqt = nc.dram_tensor("wqt", [128, NDC * KD], BF16,
                         kind="ExternalInput").ap()
    wkt = nc.dram_tensor("wkt", [128, NDC * KD], BF16,
                         kind="ExternalInput").ap()
    wvt = nc.dram_tensor("wvt", [128, NDC * KD], BF16,
                         kind="ExternalInput").ap()
    wot = nc.dram_tensor("wot", [128, NKK * D], BF16,
                         kind="ExternalInput").ap()
    maskd = nc.dram_tensor("mask", [128, 128], BF16, kind="ExternalInput").ap()
    seld = nc.dram_tensor("sel", [2, 128], F32R, kind="ExternalInput").ap()
    y = nc.dram_tensor("y", [S, D], BF16, kind="ExternalOutput").ap()

    with tile.TileContext(nc) as tc, ExitStack() as ctx:
        wpool = ctx.enter_context(tc.tile_pool(name="w", bufs=1))
        big = ctx.enter_context(tc.tile_pool(name="big", bufs=1))
        xtg_pool = ctx.enter_context(tc.tile_pool(name="xtg", bufs=32))
        e_pool = ctx.enter_context(tc.tile_pool(name="expS", bufs=6))
        sm_pool = ctx.enter_context(tc.tile_pool(name="small", bufs=2))
        ypool = ctx.enter_context(tc.tile_pool(name="yout", bufs=4))
        psS = ctx.enter_context(tc.tile_pool(name="psS", bufs=2, space="PSUM"))
        psM = ctx.enter_context(tc.tile_pool(name="psM", bufs=2, space="PSUM"))
        psO = ctx.enter_context(tc.tile_pool(name="psO", bufs=2, space="PSUM"))

        # --- persistent SBUF tensors ---
        wq_sb = wpool.tile([128, NDC, KD], BF16)
        wk_sb = wpool.tile([128, NDC, KD], BF16)
        wv_sb = wpool.tile([128, NDC, KD], BF16)
        wo_sb = wpool.tile([128, NKK, D], BF16)
        mask_sb = wpool.tile([128, 128], BF16)
        selA_sb = wpool.tile([1, 128], F32R)
        selB_sb = wpool.tile([1, 128], F32R)
        QT = big.tile([128, NKK, S], BF16)   # [channel(2 heads), kk, token]
        KT = big.tile([128, NKK, S], BF16)
        VA = big.tile([128, HPC, NUT, HS + 1], BF16)  # [tok, head, ut, ch|1]
        OT = big.tile([128, NKK, S], BF16)   # normalized attention out^T

        # DMA issue order is arrival priority: wq + wk + x(tg0) gate the
        # interleaved prelude Q/K chains; everything else trails.
        nc.sync.dma_start(wq_sb[:], wqt.rearrange("p (c k) -> p c k", c=NDC))
        nc.sync.dma_start(wk_sb[:], wkt.rearrange("p (c k) -> p c k", c=NDC))
        xts = [[None] * NTB for _ in range(NDC)]
        for tg in range(NTB):
            for c in range(NDC):
                t = xtg_pool.tile([128, TB], BF16, tag="xtg",
                                  name=f"xt{c}_{tg}")
                r0 = (c * NTB + tg) * 128
                nc.sync.dma_start(t[:], xt[r0:r0 + 128, :])
                xts[c][tg] = t
            if tg == 0:
                nc.sync.dma_start(mask_sb[:], maskd)
                nc.sync.dma_start(selA_sb[:], seld[0:1, :])
                nc.sync.dma_start(selB_sb[:], seld[1:2, :])
                nc.sync.dma_start(
                    wv_sb[:], wvt.rearrange("p (c k) -> p c k", c=NDC))
            if tg == 2:
                nc.sync.dma_start(
                    wo_sb[:], wot.rearrange("p (v d) -> p v d", v=NKK))
        nc.vector.memset(VA[:, :, :, HS], 1.0)

        # --- filler units, split into ~0.4-0.9us pieces so a single step's
        # injected PE work never exceeds the exp pacing budget. A unit's
        # PSUM tile is allocated by its first piece and released by the
        # staging copy in its last; pieces of one unit occupy consecutive
        # steps so at most two psM chains are ever open. ---
        def qk_pieces(tg, kk, w_sb, dst, on_scalar):
            st = {}

            def mk(half):
                def f():
                    if half == 0:
                        st['ps'] = psM.tile([128, TB], F32, tag="m512",
                                            name=f"qk{tg}_{kk}")
                    ps = st['ps']
                    for c in range(4 * half, 4 * half + 4):
                        nc.tensor.matmul(
                            ps[:], w_sb[:, c, 128 * kk:128 * (kk + 1)],
                            xts[c][tg][:],
                            start=(c == 0), stop=(c == NDC - 1),
                        )
                    if half == 1:
                        sl = dst[:, kk, TB * tg:TB * (tg + 1)]
                        if on_scalar:
                            nc.scalar.copy(sl, ps[:])
                        else:
                            nc.vector.tensor_copy(sl, ps[:])
                return f
            return [mk(0), mk(1)]

        def v_unit(tg, tt, on_scalar):
            ps = psM.tile([128, TB], F32, tag="m512", name=f"v{tg}_{tt}")
            for c in range(NDC):
                nc.tensor.matmul(
                    ps[:, 0:KD], xts[c][tg][:, 128 * tt:128 * (tt + 1)],
                    wv_sb[:, c, :], start=(c == 0), stop=(c == NDC - 1),
                )
            ut = (TB // 128) * tg + tt
            src = ps[:, 0:KD].rearrange("p (h c) -> p h c", h=HPC)
            if on_scalar:
                nc.scalar.copy(VA[:, :, ut, 0:HS], src)
            else:
                nc.vector.tensor_copy(VA[:, :, ut, 0:HS], src)

        def oproj_unit(tb, i, j, alt_pool=False):
            if alt_pool:
                spt = psS.tile([128, 2, TB], F32, tag="sp", name=f"y{i}_{j}")
                ps = spt[:, 0, :]
            else:
                ps = psM.tile([128, TB], F32, tag="m512", name=f"y{i}_{j}")[:]
            for vc in range(NKK):
                nc.tensor.matmul(
                    ps, OT[:, vc, 128 * i:128 * (i + 1)],
                    wo_sb[:, vc, TB * j:TB * (j + 1)],
                    start=(vc == 0), stop=(vc == NKK - 1),
                )
            yt = ypool.tile([128, TB], BF16, tag="yt")
            nc.vector.tensor_copy(yt[:], ps)
            nc.sync.dma_start(
                y[128 * i:128 * (i + 1), TB * j:TB * (j + 1)], yt[:])

        # deadline table: (tb, hp, k) -> [piece closures]
        due = {}

        def add_due(tb, hp, k, fn):
            due.setdefault((tb, hp, k), []).append(fn)

        def add_pieces(tb, hp, k_end, pieces):
            for i, p in enumerate(pieces):
                add_due(tb, hp, max(0, k_end - (len(pieces) - 1 - i)), p)

        for tg in range(NTB):
            early = tg <= 1  # staging copies on ScalarE before exp saturates
            nut_tg = 4 * tg + 4
            for kk in range(NKK):
                if (tg, kk) != (0, 0):
                    q_key = (tg, 0, 4) if kk == 1 else \
                        (tg - 1, 1, min(4 * (tg - 1) + 3, 5))
                    if (tg, kk) == (0, 1):
                        q_key = (0, 0, 2)
                    add_pieces(*q_key, qk_pieces(tg, kk, wq_sb, QT, early))
                    k_key = (0, 0, 3) if (tg, kk) == (0, 1) \
                        else (tg, kk, max(2, 4 * tg - 2))
                    add_pieces(*k_key, qk_pieces(tg, kk, wk_sb, KT, early))
            # V(tg,0) is consumed in-phase; V(tg,1..3) only by the carried
            # AV drain, which runs at (tg, 1, k=1) — spread them there.
            vdl = [(tg, 0, min(4 * tg, nut_tg - 2)), (tg, 0, nut_tg - 1),
                   (tg, 1, 0), (tg, 1, 1)]
            for tt in range(TB // 128):
                add_due(*vdl[tt],
                        lambda tg=tg, tt=tt, e=early: v_unit(tg, tt, e))
        # o_proj(0) woven into (tb2, hp1); o_proj(1)/(2) into tb3
        for idx, (i, j) in enumerate((i, j) for i in range(0, 4)
                                     for j in range(NJ)):
            add_due(2, 1, 3 + idx, lambda i=i, j=j: oproj_unit(0, i, j))
        for idx, (i, j) in enumerate((i, j) for i in range(4, 8)
                                     for j in range(NJ)):
            key = (3, 0, 6 + 2 * idx) if idx < 4 else (3, 1, 3 + (idx - 4))
            add_due(*key, lambda i=i, j=j: oproj_unit(1, i, j))
        for idx, (i, j) in enumerate((i, j) for i in range(8, 12)
                                     for j in range(NJ)):
            add_due(3, 1, 7 + idx, lambda i=i, j=j: oproj_unit(2, i, j))

        # --- prelude: interleaved first Q/K chains so scores start ASAP ---
        psq = psM.tile([128, TB], F32, tag="m512", name="q00")
        psk = psM.tile([128, TB], F32, tag="m512", name="k00")
        for c in range(NDC):
            nc.tensor.matmul(psq[:], wq_sb[:, c, 0:128], xts[c][0][:],
                             start=(c == 0), stop=(c == NDC - 1))
            nc.tensor.matmul(psk[:], wk_sb[:, c, 0:128], xts[c][0][:],
                             start=(c == 0), stop=(c == NDC - 1))
        nc.scalar.copy(QT[:, 0, 0:TB], psq[:])
        nc.scalar.copy(KT[:, 0, 0:TB], psk[:])

        # --- attention: Scalar-paced u-steps; AV drain + normalization of
        # each phase deferred into the next phase (carry) so the boundary
        # never serializes PE->Scalar->DVE->PE. ---
        carry = [None]

        def close_phase(Os, pend, av_pair, tb, hp):
            def fin():
                for i, p in enumerate(pend):
                    av_pair(*p, stop=(i == len(pend) - 1))
                dens = [sm_pool.tile([1, TB], F32R, tag=f"den{g}",
                                     name=f"den{g}") for g in range(2)]
                for g in range(2):
                    nc.vector.tensor_copy(dens[g][:], Os[g][HS:HS + 1, :])
                bc_ps = psM.tile([128, TB], F32, tag="m512", name="bc_ps")
                nc.tensor.matmul(bc_ps[:], selA_sb[:], dens[0][:],
                                 start=True, stop=False)
                nc.tensor.matmul(bc_ps[:], selB_sb[:], dens[1][:],
                                 start=False, stop=True)
                bc = sm_pool.tile([128, TB], F32, tag="bc_sb")
                nc.vector.reciprocal_approx_fast(bc[:], bc_ps[:])
                for g in range(2):
                    ro = 64 * g
                    nc.vector.tensor_mul(
                        OT[ro:ro + 64, hp, TB * tb:TB * (tb + 1)],
                        Os[g][0:HS, :], bc[ro:ro + 64, :]
                    )
            return fin

        for tb in range(NTB):
            nut = 4 * tb + 4
            for hp in range(NKK):
                Os = [psO.tile([HS + 1, TB], F32, tag="av", name=f"O{g}")
                      for g in range(2)]

                def av_pair(pes, ptoff, pk, stop, Os=Os, hp=hp):
                    for g in range(2):
                        nc.tensor.matmul(
                            Os[g][:, ptoff:],
                            VA[:, 2 * hp + g, pk, :],
                            pes[:, g, ptoff:],
                            start=(pk == 0), stop=stop,
                        )

                pend = []
                for k in range(nut):
                    # filler units first: anything scores(k) might consume
                    # (KT/QT/VA producers) must precede it in the PE queue.
                    for u in due.pop((tb, hp, k), []):
                        u()
                    toff = max(0, 128 * (k - 4 * tb))
                    sp = psS.tile([128, 2, TB], F32, tag="sp")
                    for g in range(2):
                        nc.tensor.matmul(
                            sp[:, g, toff:],
                            KT[64 * g:64 * g + 64, hp, 128 * k:128 * (k + 1)],
                            QT[64 * g:64 * g + 64, hp,
                               TB * tb + toff:TB * (tb + 1)],
                            start=True, stop=True,
                        )
                    es = e_pool.tile([128, 2, TB], BF16, tag="expS")
                    nc.scalar.activation(es[:, :, toff:], sp[:, :, toff:], EXP,
                                         scale=SCALE)
                    if k >= 4 * tb:  # diagonal: zero the u>t triangle
                        for g in range(2):
                            nc.vector.tensor_mul(
                                es[:, g, toff:toff + 128],
                                es[:, g, toff:toff + 128], mask_sb[:]
                            )
                    if k == 1 and carry[0] is not None:
                        carry[0]()
                        carry[0] = None
                    if len(pend) > 2:
                        av_pair(*pend.pop(0), stop=False)
                    pend.append((es, toff, k))
                carry[0] = close_phase(Os, pend, av_pair, tb, hp)
        carry[0]()
        # tail: last block's o_proj, alternating PSUM pools so the
        # 2-deep psM rotation doesn't serialize the drain.
        for n, (i, j) in enumerate((i, j) for i in range(12, 16)
                                   for j in range(NJ)):
            oproj_unit(3, i, j, alt_pool=(n % 2 == 1))
        assert not due, f"unemitted filler units: {list(due)}"

    nc.compile()
    return nc


def make_in_maps(x, q_w, k_w, v_w, o_w):
    x = np.asarray(x, dtype=np.float32)
    mask = np.triu(np.ones((128, 128), dtype=np.float32))  # keep where u <= t
    sel = np.zeros((2, 128), dtype=np.float32)
    sel[0, 0:64] = 1.0
    sel[1, 64:128] = 1.0
    mask_bf = mask.astype(BFNP)

    def warr(w):  # [D_rows, cols] -> [128, nchunks*cols], chunked over rows
        d, cols = w.shape
        n = d // 128
        return np.ascontiguousarray(
            w.reshape(n, 128, cols).transpose(1, 0, 2).reshape(128, n * cols)
        ).astype(BFNP)

    xtb = []
    for b in range(B):
        xt = np.ascontiguousarray(x[b].T)  # [D, S]
        t = xt.reshape(NDC, 128, NTB, TB).transpose(0, 2, 1, 3)
        xtb.append(np.ascontiguousarray(
            t.reshape(NDC * NTB * 128, TB)).astype(BFNP))

    in_maps = []
    for c in range(NCORES):
        b, hg = divmod(c, NCORES // B)
        sl = slice(hg * KD, (hg + 1) * KD)
        in_maps.append({
            "xt": xtb[b],
            "wqt": warr(np.ascontiguousarray(np.asarray(q_w, np.float32)[sl, :].T)),
            "wkt": warr(np.ascontiguousarray(np.asarray(k_w, np.float32)[sl, :].T)),
            "wvt": warr(np.ascontiguousarray(np.asarray(v_w, np.float32)[sl, :].T)),
            "wot": warr(np.ascontiguousarray(np.asarray(o_w, np.float32)[:, sl].T)),
            "mask": mask_bf,
            "sel": sel,
        })
    return in_maps


def combine_outputs(results):
    """results: list of 8 dicts with per-core partial y [S, D]."""
    per_b = NCORES // B
    ys = [np.asarray(results[c]["y"], dtype=np.float32) for c in range(NCORES)]
    out = np.stack(
        [sum(ys[b * per_b + i] for i in range(per_b)) for b in range(B)]
    )
    return np.ascontiguousarray(out, dtype=np.float32)


_PROGRAM = None


def kernel(x, q_proj_weight, k_proj_weight, v_proj_weight, o_proj_weight,
           **extra):
    global _PROGRAM
    if _PROGRAM is None:
        _PROGRAM = build_program()
    in_maps = make_in_maps(x, q_proj_weight, k_proj_weight, v_proj_weight,
                           o_proj_weight)
    res = run_bass_kernel_spmd(_PROGRAM, in_maps, list(range(NCORES)))
    return combine_outputs(res.results)


if __name__ == "__main__":
    nc = build_program()
    print("program built")
